# revision 5
# baseline (speedup 1.0000x reference)
"""Multi-head causal attention (RoPE + per-head RMSNorm) on 8 TRN2 NeuronCores.

v2: activation-table churn removed (Ln/Exp only on ACT engine), k-norm folded
into exp's per-partition scale AP, denominator via fp32 reciprocal + exp bias
shrink, partial-column diagonal tiles, software-pipelined emission that keeps
the in-order PE queue fed (score(si+1) before AV(si), next-chunk projections
and prev-chunk output projection interleaved at si granularity).

Sharding: core c -> batch b = c//2, head group g = c%2 (heads 8g..8g+8).
Each core computes a partial out[b] over its 8 heads' channels; host sums the
two partials per batch.
"""

import sys

import numpy as np

sys.path.insert(0, "/opt/trn_rl_repo")

B, T, C, H, D = 4, 2048, 1024, 16, 64
N_CORES = 8
HPC = H // 2  # heads per core: 8
TC = 512  # t-chunk (matmul free dim)
NTC = T // TC  # 4
NST = T // 128  # 16 s/t subtiles
LOG_SHRINK = -6.931471805599453  # ln(2^-10): at' shrink so unnormalized y fits fp16

_STATE: dict = {}


def _force_combined_exp_ln_table():
    """Make the act-table chooser pick natural_log_exp_and_others for both
    Exp and Ln (greedy first-match otherwise alternates between the exp-only
    and ln-only tables, costing a 1283ns table reload per switch). Table list
    positions are preserved; only the claimed function sets shrink."""
    if _STATE.get("tables_patched"):
        return
    import concourse.bacc as bacc
    import concourse.mybir as mybir

    orig = bacc.get_activation_tables
    keep = "natural_log_exp_and_others"
    drop = {mybir.ActivationFunctionType.Exp, mybir.ActivationFunctionType.Ln}

    def patched(arch):
        tabs = orig(arch)
        return {
            name: (funcs if name == keep else funcs - drop)
            for name, funcs in tabs.items()
        }

    bacc.get_activation_tables = patched
    _STATE["tables_patched"] = True


def _build_nc(loop_n=None):
    _force_combined_exp_ln_table()
    import concourse.mybir as mybir
    from concourse import bacc
    from concourse.tile import TileContext
    from contextlib import ExitStack

    f16 = mybir.dt.float16
    f32 = mybir.dt.float32
    AF = mybir.ActivationFunctionType

    nc = bacc.Bacc(
        "TRN2",
        target_bir_lowering=False,
        debug=False,
        num_devices=N_CORES,
    )

    xT = nc.dram_tensor("xT", [NTC, 128, 8, TC], f16, kind="ExternalInput")
    wqT = nc.dram_tensor("wqT", [128, 8, 512], f16, kind="ExternalInput")
    wkT = nc.dram_tensor("wkT", [128, 8, 512], f16, kind="ExternalInput")
    wvT = nc.dram_tensor("wvT", [128, 8, 512], f16, kind="ExternalInput")
    wpT = nc.dram_tensor("wpT", [128, 4, 1024], f16, kind="ExternalInput")
    cosq = nc.dram_tensor("cosq", [128, T], f16, kind="ExternalInput")
    sinq = nc.dram_tensor("sinq", [128, T], f16, kind="ExternalInput")
    cosk = nc.dram_tensor("cosk", [128, T], f16, kind="ExternalInput")
    sink = nc.dram_tensor("sink", [128, T], f16, kind="ExternalInput")
    trid = nc.dram_tensor("trid", [128, 128], f16, kind="ExternalInput")
    p2d = nc.dram_tensor("p2d", [128, 128], f16, kind="ExternalInput")
    ocd = nc.dram_tensor("ocd", [128, 2], f16, kind="ExternalInput")
    obqd = nc.dram_tensor("obqd", [2, 128], f16, kind="ExternalInput")
    outd = nc.dram_tensor("out", [T, C], f16, kind="ExternalOutput")

    with TileContext(nc) as tc, ExitStack() as ctx:
        const = ctx.enter_context(tc.tile_pool(name="const", bufs=1))
        xpool = ctx.enter_context(tc.tile_pool(name="xp", bufs=2))
        persist = ctx.enter_context(tc.tile_pool(name="persist", bufs=1))
        work = ctx.enter_context(tc.tile_pool(name="work", bufs=4))
        attp = ctx.enter_context(tc.tile_pool(name="attp", bufs=6))
        outp = ctx.enter_context(tc.tile_pool(name="outp", bufs=2))
        psA = ctx.enter_context(tc.tile_pool(name="psA", bufs=2, space="PSUM"))
        psB = ctx.enter_context(tc.tile_pool(name="psB", bufs=3, space="PSUM"))
        psY = ctx.enter_context(tc.tile_pool(name="psY", bufs=2, space="PSUM"))
        psS = ctx.enter_context(tc.tile_pool(name="psS", bufs=1, space="PSUM"))

        # ---- constants (DMA'd in first-use order) ----
        wq_sb = const.tile([128, 8, 512], f16, tag="wq")
        wk_sb = const.tile([128, 8, 512], f16, tag="wk")
        for ci in range(8):  # split so the first matmuls start at 1/8 loaded
            nc.sync.dma_start(wq_sb[:, ci, :], wqT[:, ci, :])
            nc.sync.dma_start(wk_sb[:, ci, :], wkT[:, ci, :])
        cosq_sb = const.tile([128, T], f16, tag="cosq")
        nc.sync.dma_start(cosq_sb, cosq[:, :])
        sinq_sb = const.tile([128, T], f16, tag="sinq")
        nc.sync.dma_start(sinq_sb, sinq[:, :])
        cosk_sb = const.tile([128, T], f16, tag="cosk")
        nc.sync.dma_start(cosk_sb, cosk[:, :])
        sink_sb = const.tile([128, T], f16, tag="sink")
        nc.sync.dma_start(sink_sb, sink[:, :])
        p2_sb = const.tile([128, 128], f16, tag="p2")
        nc.sync.dma_start(p2_sb, p2d[:, :])
        oc_sb = const.tile([128, 2], f16, tag="oc")
        nc.sync.dma_start(oc_sb, ocd[:, :])
        obq_sb = const.tile([2, 128], f16, tag="obq")
        nc.sync.dma_start(obq_sb, obqd[:, :])
        wv_sb = const.tile([128, 8, 512], f16, tag="wv")
        nc.sync.dma_start(wv_sb, wvT[:, :, :])
        tri_sb = const.tile([128, 128], f16, tag="tri")
        nc.sync.dma_start(tri_sb, trid[:, :])
        wp_sb = const.tile([128, 4, 1024], f16, tag="wp")
        nc.sync.dma_start(wp_sb, wpT[:, :, :])
        bsh = const.tile([128, 1], f32, tag="bsh")
        nc.vector.memset(bsh, LOG_SHRINK)

        # ---- persistent activations ----
        qT = [
            persist.tile([128, T], f16, tag=f"qT{p}", name=f"qT{p}")
            for p in range(4)
        ]
        kT = [
            persist.tile([128, T], f16, tag=f"kT{p}", name=f"kT{p}")
            for p in range(4)
        ]
        yT = [
            persist.tile([128, T], f16, tag=f"yT{p}", name=f"yT{p}")
            for p in range(4)
        ]
        v_sb = persist.tile([128, NST, HPC, 65], f16, tag="v")
        nc.vector.memset(v_sb[:, :, :, 64:65], 1.0)
        # rkT[s_part, si, h] = ss_k^-1/2 (= 0.125/rms_k); f32 for exp scale AP
        rkT = persist.tile([128, NST, HPC], f32, tag="rkT")

        xts = {}

        def dma_x(tci, split=False):
            xt = xpool.tile([128, 8, TC], f16, tag="x", name=f"xt{tci}")
            if split:
                for ci in range(8):
                    nc.sync.dma_start(xt[:, ci, :], xT[tci, :, ci, :])
            else:
                nc.sync.dma_start(xt, xT[tci])
            xts[tci] = xt

        def gen_a_pair(tci, p):
            """Projection + RoPE + stats for head pair p of chunk tci.
            Yields after each atomic step (~1 PE matmul of work)."""
            xt = xts[tci]
            tsl = slice(tci * TC, (tci + 1) * TC)
            ps_q = psA.tile([128, TC], f32, tag="kqv", name="ps_q")
            for ci in range(8):
                nc.tensor.matmul(
                    ps_q,
                    lhsT=wq_sb[:, ci, p * 128 : (p + 1) * 128],
                    rhs=xt[:, ci, :],
                    start=(ci == 0),
                    stop=(ci == 7),
                )
                yield
            qraw = work.tile([128, TC], f16, tag="qraw")
            nc.vector.tensor_copy(qraw, ps_q)
            sq_q = work.tile([128, TC], f16, tag="sq_q")
            nc.vector.tensor_mul(sq_q, qraw, qraw)
            yield
            # q stats: colsums of sq_q per head -> [2, TC]
            ss = psS.tile([128, TC], f32, tag="s", name="ss_q")
            nc.tensor.matmul(ss[0:2, :], lhsT=oc_sb, rhs=sq_q, start=True, stop=True)
            # q norm scalars: rr_q = (ss_q/64)^-0.5 via Ln+Exp, f16 for bc matmul
            lnq = work.tile([2, TC], f32, tag="lnq")
            nc.scalar.activation(lnq, ss[0:2, :], AF.Ln, scale=1.0 / 64.0)
            rrq = work.tile([2, TC], f16, tag="rrq")
            with nc.allow_low_precision(reason="1/rms ~1 fits fp16"):
                nc.scalar.activation(rrq, lnq, AF.Exp, scale=-0.5)
            yield
            bc = psS.tile([128, TC], f32, tag="s", name="bc_q")
            nc.tensor.matmul(bc, lhsT=obq_sb, rhs=rrq, start=True, stop=True)
            yield
            # RoPE q (w_q folded into cosq/sinq tables)
            rot_q = psA.tile([128, TC], f32, tag="kqv", name="rot_q")
            nc.tensor.matmul(rot_q, lhsT=p2_sb, rhs=qraw, start=True, stop=True)
            yield
            t1q = work.tile([128, TC], f16, tag="t1q")
            nc.vector.tensor_mul(t1q, qraw, cosq_sb[:, tsl])
            qshq = work.tile([128, TC], f16, tag="qshq")
            nc.vector.tensor_mul(qshq, rot_q, sinq_sb[:, tsl])
            t2q = work.tile([128, TC], f16, tag="t2q")
            nc.vector.tensor_add(t2q, t1q, qshq)
            nc.vector.tensor_mul(qT[p][:, tsl], t2q, bc)
            yield
            ps_k = psA.tile([128, TC], f32, tag="kqv", name="ps_k")
            for ci in range(8):
                nc.tensor.matmul(
                    ps_k,
                    lhsT=wk_sb[:, ci, p * 128 : (p + 1) * 128],
                    rhs=xt[:, ci, :],
                    start=(ci == 0),
                    stop=(ci == 7),
                )
                yield
            kraw = work.tile([128, TC], f16, tag="kraw")
            nc.vector.tensor_copy(kraw, ps_k)
            sq_k = work.tile([128, TC], f16, tag="sq_k")
            nc.vector.tensor_mul(sq_k, kraw, kraw)
            yield
            # k stats, transposed: ssT[s_part, st, h] via 4 tiny matmuls
            ss2 = psS.tile([128, TC], f32, tag="s", name="ss_k")
            for st in range(4):
                nc.tensor.matmul(
                    ss2[:, 2 * st : 2 * st + 2],
                    lhsT=sq_k[:, st * 128 : (st + 1) * 128],
                    rhs=oc_sb,
                    start=True,
                    stop=True,
                )
            # rkT = ss_k^-0.5 = exp(-0.5 ln(ss_k))  [0.125/sqrt(64) folded exactly]
            lnk = work.tile([128, 4, 2], f32, tag="lnk")
            nc.scalar.activation(
                lnk, ss2[:, 0:8].rearrange("p (a b) -> p a b", b=2), AF.Ln
            )
            nc.scalar.activation(
                rkT[:, tci * 4 : tci * 4 + 4, 2 * p : 2 * p + 2],
                lnk,
                AF.Exp,
                scale=-0.5,
            )
            yield
            # RoPE k (no norm here; k-norm applied via exp scale)
            rot_k = psA.tile([128, TC], f32, tag="kqv", name="rot_k")
            nc.tensor.matmul(rot_k, lhsT=p2_sb, rhs=kraw, start=True, stop=True)
            yield
            t1k = work.tile([128, TC], f16, tag="t1k")
            nc.vector.tensor_mul(t1k, kraw, cosk_sb[:, tsl])
            qshk = work.tile([128, TC], f16, tag="qshk")
            nc.vector.tensor_mul(qshk, rot_k, sink_sb[:, tsl])
            nc.vector.tensor_add(kT[p][:, tsl], t1k, qshk)
            yield

        def gen_a_v(tci, st):
            """v projection for subtile st of chunk tci."""
            xt = xts[tci]
            pv = psA.tile([128, TC], f32, tag="kqv", name="pv")
            for ci in range(8):
                nc.tensor.matmul(
                    pv,
                    lhsT=xt[:, ci, st * 128 : (st + 1) * 128],
                    rhs=wv_sb[:, ci, :],
                    start=(ci == 0),
                    stop=(ci == 7),
                )
                yield
            nc.vector.tensor_copy(
                v_sb[:, tci * 4 + st, :, 0:64],
                pv.rearrange("p (h d) -> p h d", h=HPC),
            )
            yield

        def gen_c_unit(tci, st_local, co):
            """Output projection for token tile st of chunk tci, half co."""
            st = tci * 4 + st_local
            po = psA.tile([128, TC], f32, tag="kqv", name="po")
            for p in range(4):
                nc.tensor.matmul(
                    po,
                    lhsT=yT[p][:, st * 128 : (st + 1) * 128],
                    rhs=wp_sb[:, p, co * 512 : (co + 1) * 512],
                    start=(p == 0),
                    stop=(p == 3),
                )
                yield
            ot = outp.tile([128, TC], f16, tag="o")
            with nc.allow_low_precision(reason="fp16 partials; host sums in fp32"):
                nc.vector.tensor_copy(ot, po)
            nc.sync.dma_start(
                outd[st * 128 : (st + 1) * 128, co * 512 : (co + 1) * 512], ot
            )
            yield

        def emit_b_head(tci, h, pop):
            """Attention for head h over this t-chunk; pop() drains filler."""
            p, hl = h // 2, h % 2
            hsl = slice(hl * 64, (hl + 1) * 64)
            n_s = 4 * (tci + 1)
            tbase = tci * TC
            tsl = slice(tbase, tbase + TC)

            def score(si):
                d = si * 128 - tbase if si >= 4 * tci else 0
                ps_s = psB.tile([128, TC], f32, tag="sc", name="ps_s")
                nc.tensor.matmul(
                    ps_s[:, d:TC],
                    lhsT=kT[p][hsl, si * 128 : (si + 1) * 128],
                    rhs=qT[p][hsl, tbase + d : tbase + TC],
                    start=True,
                    stop=True,
                )
                return ps_s, d

            ps_y = psY.tile([128, TC], f32, tag="y", name="ps_y")
            pend = [score(0)]
            if n_s > 1:
                pend.append(score(1))
            for si in range(n_s):
                ps_s, d = pend.pop(0)
                at = attp.tile([128, TC], f16, tag="at")
                with nc.allow_low_precision(reason="softmax weights fp16"):
                    nc.scalar.activation(
                        at[:, d:TC],
                        ps_s[:, d:TC],
                        AF.Exp,
                        bias=bsh,
                        scale=rkT[:, si, h : h + 1],
                    )
                if si + 2 < n_s:
                    pend.append(score(si + 2))
                if si >= 4 * tci:  # diagonal subtile: triangular mask block
                    nc.vector.tensor_mul(
                        at[:, d : d + 128], at[:, d : d + 128], tri_sb
                    )
                nc.tensor.matmul(
                    ps_y[0:65, d:TC],
                    lhsT=v_sb[:, si, h, :],
                    rhs=at[:, d:TC],
                    start=(si == 0),
                    stop=(si == n_s - 1),
                )
                pop()
            # softmax denominator: row 64 of ps_y (shrunk by 2^-10, cancels)
            rec = work.tile([1, TC], f16, tag="rec")
            with nc.allow_low_precision(reason="1/denom bounded by shrink"):
                nc.vector.reciprocal(rec, ps_y[64:65, :])
            db_sb = work.tile([64, TC], f16, tag="db")
            nc.gpsimd.partition_broadcast(db_sb, rec)
            nc.vector.tensor_mul(yT[p][hsl, tsl], ps_y[0:64, :], db_sb)
            pop()

        def body():
            # prologue: x chunk 0 + projections for chunk 0
            dma_x(0, split=True)
            for p in range(4):
                for _ in gen_a_pair(0, p):
                    pass
            for st in range(4):
                for _ in gen_a_v(0, st):
                    pass
            dma_x(1)

            from itertools import chain

            N_PAIR_STEPS, N_V_STEPS, N_C_STEPS = 25, 9, 5
            for tci in range(NTC):
                if tci + 2 < NTC:
                    dma_x(tci + 2)
                gens = []
                total_fill = 0
                if tci + 1 < NTC:
                    for p in range(4):
                        gens.append(gen_a_pair(tci + 1, p))
                        total_fill += N_PAIR_STEPS
                    for st in range(4):
                        gens.append(gen_a_v(tci + 1, st))
                        total_fill += N_V_STEPS
                if tci > 0:
                    for st_local in range(4):
                        for co in range(2):
                            gens.append(gen_c_unit(tci - 1, st_local, co))
                            total_fill += N_C_STEPS
                fill_it = chain(*gens)

                steps_total = 8 * (4 * (tci + 1) + 1)
                state = {"step": 0, "emitted": 0}

                def pop():
                    state["step"] += 1
                    target = (state["step"] * total_fill + steps_total - 1) // steps_total
                    while state["emitted"] < target:
                        try:
                            next(fill_it)
                        except StopIteration:
                            state["emitted"] = total_fill
                            break
                        state["emitted"] += 1

                for h in range(HPC):
                    emit_b_head(tci, h, pop)
                # drain leftover filler
                for _ in fill_it:
                    pass

            # epilogue: output projection for last chunk
            for st_local in range(4):
                for co in range(2):
                    for _ in gen_c_unit(NTC - 1, st_local, co):
                        pass

        if loop_n is None:
            body()
        else:
            with tc.For_i(0, loop_n, 1):
                body()

    return nc


def _get_nc(loop_n=None):
    key = ("nc", loop_n)
    if key not in _STATE:
        nc = _build_nc(loop_n)
        nc.finalize()
        _STATE[key] = nc
    return _STATE[key]


def _rope_tables(w):
    """cosW/sinW [128, T] f16 with per-dim norm weight folded in."""
    inv_freq = 1.0 / (10000.0 ** (np.arange(0, D, 2, dtype=np.float64) / D))
    t_pos = np.arange(T, dtype=np.float64)
    freqs = t_pos[:, None] * inv_freq[None, :]  # [T, 32]
    f2 = np.concatenate([freqs, freqs], axis=-1)  # [T, 64]
    w = np.asarray(w, dtype=np.float64)[:, None]  # [64, 1]
    cosT = (w * np.cos(f2).T).astype(np.float16)  # [64, T]
    sinT = (w * np.sin(f2).T).astype(np.float16)
    cos2 = np.concatenate([cosT, cosT], axis=0)  # [128, T]
    sin2 = np.concatenate([sinT, sinT], axis=0)
    return np.ascontiguousarray(cos2), np.ascontiguousarray(sin2)


def _prep_inputs(x, W_kqv, W_proj, q_norm_w, k_norm_w):
    x = np.asarray(x, dtype=np.float32)
    W_kqv = np.asarray(W_kqv, dtype=np.float32)
    W_proj = np.asarray(W_proj, dtype=np.float32)
    q_norm_w = np.asarray(q_norm_w, dtype=np.float32)
    k_norm_w = np.asarray(k_norm_w, dtype=np.float32)

    cosq, sinq = _rope_tables(q_norm_w)
    cosk, sink = _rope_tables(k_norm_w)

    # triangular mask for the diagonal 128-block: keep t >= s
    si = np.arange(128)[:, None]
    cj = np.arange(128)[None, :]
    tri = (cj >= si).astype(np.float16)

    # oc: colsum selectors per head half
    oc = np.zeros((128, 2), dtype=np.float16)
    oc[0:64, 0] = 1.0
    oc[64:128, 1] = 1.0

    # obq: expand rr rows to 64-partition halves (pure ones; w in tables)
    obq = np.zeros((2, 128), dtype=np.float16)
    obq[0, 0:64] = 1.0
    obq[1, 64:128] = 1.0

    # signed rotate-half permutation (per 64-dim head, stacked twice)
    P = np.zeros((64, 64), dtype=np.float16)
    for i in range(32):
        P[i, i + 32] = -1.0
        P[i + 32, i] = 1.0
    P2 = np.zeros((128, 128), dtype=np.float16)
    P2[0:64, 0:64] = P
    P2[64:128, 64:128] = P
    p2T = np.ascontiguousarray(P2.T)

    def wt_kqv(rows):
        # rows: [512, 1024] -> lhsT layout [128, 8, 512] fp16
        wT = rows.T.astype(np.float16)  # [1024, 512]
        return np.ascontiguousarray(wT.reshape(8, 128, 512).transpose(1, 0, 2))

    Wk, Wq, Wv = W_kqv[0:C], W_kqv[C : 2 * C], W_kqv[2 * C : 3 * C]

    in_maps = []
    for c in range(N_CORES):
        b, g = c // 2, c % 2
        rs = slice(512 * g, 512 * (g + 1))
        xTb = x[b].T.astype(np.float16)  # [C, T]
        xTr = np.ascontiguousarray(
            xTb.reshape(8, 128, NTC, TC).transpose(2, 1, 0, 3)
        )  # [NTC, 128, 8, TC]
        wp = W_proj[:, rs].T.astype(np.float16)  # [512, 1024]
        wpr = np.ascontiguousarray(wp.reshape(4, 128, 1024).transpose(1, 0, 2))
        in_maps.append(
            {
                "xT": xTr,
                "wqT": wt_kqv(Wq[rs]),
                "wkT": wt_kqv(Wk[rs]),
                "wvT": wt_kqv(Wv[rs]),
                "wpT": wpr,
                "cosq": cosq,
                "sinq": sinq,
                "cosk": cosk,
                "sink": sink,
                "trid": tri,
                "p2d": p2T,
                "ocd": oc,
                "obqd": obq,
            }
        )
    return in_maps


def _get_runner(loop_n=None):
    """Build (once) a cached jitted SPMD runner mirroring
    bass2jax.run_bass_via_pjrt, so repeated calls reuse the compiled NEFF."""
    key = ("runner", loop_n)
    if key in _STATE:
        return _STATE[key]

    import jax
    import concourse.mybir as mybir
    from concourse import bass2jax
    from concourse.bass2jax import _bass_exec_p, partition_id_tensor
    from jax.experimental.shard_map import shard_map
    from jax.sharding import Mesh, NamedSharding, PartitionSpec

    bass2jax.install_neuronx_cc_hook()
    nc = _get_nc(loop_n)

    partition_name = nc.partition_id_tensor.name if nc.partition_id_tensor else None
    in_names, out_names, out_avals, zero_outs = [], [], [], []
    for alloc in nc.m.functions[0].allocations:
        if not isinstance(alloc, mybir.MemoryLocationSet):
            continue
        name = alloc.memorylocations[0].name
        if alloc.kind == "ExternalInput":
            if name != partition_name:
                in_names.append(name)
        elif alloc.kind == "ExternalOutput":
            shape = tuple(alloc.tensor_shape)
            dtype = mybir.dt.np(alloc.dtype)
            out_names.append(name)
            out_avals.append(jax.core.ShapedArray(shape, dtype))
            zero_outs.append(np.zeros(shape, dtype))
    n_params = len(in_names)
    all_names = in_names + out_names
    if partition_name is not None:
        all_names.append(partition_name)

    def _body(*args):
        operands = list(args)
        if partition_name is not None:
            operands.append(partition_id_tensor())
        outs = _bass_exec_p.bind(
            *operands,
            out_avals=tuple(out_avals),
            in_names=tuple(all_names),
            out_names=tuple(out_names),
            lowering_input_output_aliases=(),
            sim_require_finite=True,
            sim_require_nnan=True,
            nc=nc,
        )
        return tuple(outs)

    devices = jax.devices()[:N_CORES]
    mesh = Mesh(np.asarray(devices), ("core",))
    spec = PartitionSpec("core")
    n_outs = len(out_names)
    sharded = jax.jit(
        shard_map(
            _body,
            mesh=mesh,
            in_specs=(spec,) * (n_params + n_outs),
            out_specs=(spec,) * n_outs,
            check_rep=False,
        ),
        keep_unused=True,
    )
    sharding = NamedSharding(mesh, spec)
    zeros_dev = [
        jax.device_put(
            np.zeros((N_CORES * z.shape[0], *z.shape[1:]), z.dtype), sharding
        )
        for z in zero_outs
    ]
    runner = {
        "sharded": sharded,
        "in_names": in_names,
        "out_names": out_names,
        "out_avals": out_avals,
        "zeros_dev": zeros_dev,
        "sharding": sharding,
    }
    _STATE[key] = runner
    return runner


def _concat_inputs(in_maps, runner):
    return [
        np.concatenate([np.asarray(in_maps[c][n]) for c in range(N_CORES)], axis=0)
        for n in runner["in_names"]
    ]


def _execute(in_maps):
    """Returns list (per core) of {out_name: np.ndarray}."""
    runner = _get_runner()
    concat_in = _concat_inputs(in_maps, runner)
    out_arrs = runner["sharded"](*concat_in, *runner["zeros_dev"])
    return [
        {
            n: np.asarray(out_arrs[i]).reshape(
                N_CORES, *runner["out_avals"][i].shape
            )[c]
            for i, n in enumerate(runner["out_names"])
        }
        for c in range(N_CORES)
    ]


def _wall(runner, in_maps, iters):
    import time
    import jax

    concat_in = [
        jax.device_put(a, runner["sharding"])
        for a in _concat_inputs(in_maps, runner)
    ]
    args = (*concat_in, *runner["zeros_dev"])
    jax.block_until_ready(runner["sharded"](*args))  # warmup
    times = []
    for _ in range(iters):
        t0 = time.perf_counter()
        jax.block_until_ready(runner["sharded"](*args))
        times.append(time.perf_counter() - t0)
    times.sort()
    return times


def _timed(in_maps, iters=20, n_lo=1, n_hi=33):
    """Per-pass HW time via two device-side repeat counts: the dispatch/tunnel
    overhead cancels in the difference."""
    r_lo = _get_runner(None if n_lo == 1 else n_lo)
    r_hi = _get_runner(n_hi)
    t_lo = _wall(r_lo, in_maps, iters)
    t_hi = _wall(r_hi, in_maps, iters)
    k = max(3, iters // 4)
    lo = sum(t_lo[:k]) / k
    hi = sum(t_hi[:k]) / k
    per_pass = (hi - lo) / (n_hi - n_lo)
    return per_pass, lo, hi


def kernel(**inputs):
    in_maps = _prep_inputs(**inputs)
    res = _execute(in_maps)
    out = np.zeros((B, T, C), dtype=np.float32)
    for c in range(N_CORES):
        out[c // 2] += res[c]["out"].astype(np.float32)
    return out


# revision 6
# speedup vs baseline: 1.0307x; 1.0307x over previous
"""Multi-head causal attention (RoPE + per-head RMSNorm) on 8 TRN2 NeuronCores.

v2: activation-table churn removed (Ln/Exp only on ACT engine), k-norm folded
into exp's per-partition scale AP, denominator via fp32 reciprocal + exp bias
shrink, partial-column diagonal tiles, software-pipelined emission that keeps
the in-order PE queue fed (score(si+1) before AV(si), next-chunk projections
and prev-chunk output projection interleaved at si granularity).

Sharding: core c -> batch b = c//2, head group g = c%2 (heads 8g..8g+8).
Each core computes a partial out[b] over its 8 heads' channels; host sums the
two partials per batch.
"""

import sys

import numpy as np

sys.path.insert(0, "/opt/trn_rl_repo")

B, T, C, H, D = 4, 2048, 1024, 16, 64
N_CORES = 8
HPC = H // 2  # heads per core: 8
TC = 512  # t-chunk (matmul free dim)
NTC = T // TC  # 4
NST = T // 128  # 16 s/t subtiles
LOG_SHRINK = -6.931471805599453  # ln(2^-10): at' shrink so unnormalized y fits fp16

_STATE: dict = {}


def _force_combined_exp_ln_table():
    """Make the act-table chooser pick natural_log_exp_and_others for both
    Exp and Ln (greedy first-match otherwise alternates between the exp-only
    and ln-only tables, costing a 1283ns table reload per switch). Table list
    positions are preserved; only the claimed function sets shrink."""
    if _STATE.get("tables_patched"):
        return
    import concourse.bacc as bacc
    import concourse.mybir as mybir

    orig = bacc.get_activation_tables
    keep = "natural_log_exp_and_others"
    drop = {mybir.ActivationFunctionType.Exp, mybir.ActivationFunctionType.Ln}

    def patched(arch):
        tabs = orig(arch)
        return {
            name: (funcs if name == keep else funcs - drop)
            for name, funcs in tabs.items()
        }

    bacc.get_activation_tables = patched
    _STATE["tables_patched"] = True


def _build_nc(loop_n=None):
    _force_combined_exp_ln_table()
    import concourse.mybir as mybir
    from concourse import bacc
    from concourse.tile import TileContext
    from contextlib import ExitStack

    f16 = mybir.dt.float16
    f32 = mybir.dt.float32
    AF = mybir.ActivationFunctionType

    nc = bacc.Bacc(
        "TRN2",
        target_bir_lowering=False,
        debug=False,
        num_devices=N_CORES,
    )

    xT = nc.dram_tensor("xT", [NTC, 128, 8, TC], f16, kind="ExternalInput")
    wqT = nc.dram_tensor("wqT", [128, 8, 512], f16, kind="ExternalInput")
    wkT = nc.dram_tensor("wkT", [128, 8, 512], f16, kind="ExternalInput")
    wvT = nc.dram_tensor("wvT", [128, 8, 512], f16, kind="ExternalInput")
    wpT = nc.dram_tensor("wpT", [128, 4, 1024], f16, kind="ExternalInput")
    cosq = nc.dram_tensor("cosq", [128, T], f16, kind="ExternalInput")
    sinq = nc.dram_tensor("sinq", [128, T], f16, kind="ExternalInput")
    cosk = nc.dram_tensor("cosk", [128, T], f16, kind="ExternalInput")
    sink = nc.dram_tensor("sink", [128, T], f16, kind="ExternalInput")
    trid = nc.dram_tensor("trid", [128, 128], f16, kind="ExternalInput")
    p2d = nc.dram_tensor("p2d", [128, 128], f16, kind="ExternalInput")
    ocd = nc.dram_tensor("ocd", [128, 2], f16, kind="ExternalInput")
    obqd = nc.dram_tensor("obqd", [2, 128], f16, kind="ExternalInput")
    outd = nc.dram_tensor("out", [T, C], f16, kind="ExternalOutput")

    with TileContext(nc) as tc, ExitStack() as ctx:
        const = ctx.enter_context(tc.tile_pool(name="const", bufs=1))
        xpool = ctx.enter_context(tc.tile_pool(name="xp", bufs=2))
        persist = ctx.enter_context(tc.tile_pool(name="persist", bufs=1))
        work = ctx.enter_context(tc.tile_pool(name="work", bufs=4))
        attp = ctx.enter_context(tc.tile_pool(name="attp", bufs=6))
        outp = ctx.enter_context(tc.tile_pool(name="outp", bufs=2))
        psA = ctx.enter_context(tc.tile_pool(name="psA", bufs=2, space="PSUM"))
        psB = ctx.enter_context(tc.tile_pool(name="psB", bufs=2, space="PSUM"))
        psY = ctx.enter_context(tc.tile_pool(name="psY", bufs=2, space="PSUM"))
        psS = ctx.enter_context(tc.tile_pool(name="psS", bufs=2, space="PSUM"))

        # ---- constants (DMA'd in first-use order) ----
        wq_sb = const.tile([128, 8, 512], f16, tag="wq")
        wk_sb = const.tile([128, 8, 512], f16, tag="wk")
        for ci in range(8):  # split so the first matmuls start at 1/8 loaded
            nc.sync.dma_start(wq_sb[:, ci, :], wqT[:, ci, :])
            nc.sync.dma_start(wk_sb[:, ci, :], wkT[:, ci, :])
        cosq_sb = const.tile([128, T], f16, tag="cosq")
        nc.sync.dma_start(cosq_sb, cosq[:, :])
        sinq_sb = const.tile([128, T], f16, tag="sinq")
        nc.sync.dma_start(sinq_sb, sinq[:, :])
        cosk_sb = const.tile([128, T], f16, tag="cosk")
        nc.sync.dma_start(cosk_sb, cosk[:, :])
        sink_sb = const.tile([128, T], f16, tag="sink")
        nc.sync.dma_start(sink_sb, sink[:, :])
        p2_sb = const.tile([128, 128], f16, tag="p2")
        nc.sync.dma_start(p2_sb, p2d[:, :])
        oc_sb = const.tile([128, 2], f16, tag="oc")
        nc.sync.dma_start(oc_sb, ocd[:, :])
        obq_sb = const.tile([2, 128], f16, tag="obq")
        nc.sync.dma_start(obq_sb, obqd[:, :])
        wv_sb = const.tile([128, 8, 512], f16, tag="wv")
        nc.sync.dma_start(wv_sb, wvT[:, :, :])
        tri_sb = const.tile([128, 128], f16, tag="tri")
        nc.sync.dma_start(tri_sb, trid[:, :])
        wp_sb = const.tile([128, 4, 1024], f16, tag="wp")
        nc.sync.dma_start(wp_sb, wpT[:, :, :])
        bsh = const.tile([128, 1], f32, tag="bsh")
        nc.vector.memset(bsh, LOG_SHRINK)

        # ---- persistent activations ----
        qT = [
            persist.tile([128, T], f16, tag=f"qT{p}", name=f"qT{p}")
            for p in range(4)
        ]
        kT = [
            persist.tile([128, T], f16, tag=f"kT{p}", name=f"kT{p}")
            for p in range(4)
        ]
        yT = [
            persist.tile([128, T], f16, tag=f"yT{p}", name=f"yT{p}")
            for p in range(4)
        ]
        v_sb = persist.tile([128, NST, HPC, 65], f16, tag="v")
        nc.vector.memset(v_sb[:, :, :, 64:65], 1.0)
        # rkT[s_part, si, h] = ss_k^-1/2 (= 0.125/rms_k); f32 for exp scale AP
        rkT = persist.tile([128, NST, HPC], f32, tag="rkT")

        xts = {}

        def dma_x(tci, split=False):
            xt = xpool.tile([128, 8, TC], f16, tag="x", name=f"xt{tci}")
            if split:
                for ci in range(8):
                    nc.sync.dma_start(xt[:, ci, :], xT[tci, :, ci, :])
            else:
                nc.sync.dma_start(xt, xT[tci])
            xts[tci] = xt

        def gen_a_pair(tci, p):
            """Projection + RoPE + stats for head pair p of chunk tci.
            Yields after each atomic step (~1 PE matmul of work)."""
            xt = xts[tci]
            tsl = slice(tci * TC, (tci + 1) * TC)
            ps_q = psA.tile([128, TC], f32, tag="kqv", name="ps_q")
            for ci in range(8):
                nc.tensor.matmul(
                    ps_q,
                    lhsT=wq_sb[:, ci, p * 128 : (p + 1) * 128],
                    rhs=xt[:, ci, :],
                    start=(ci == 0),
                    stop=(ci == 7),
                )
                yield
            qraw = work.tile([128, TC], f16, tag="qraw")
            nc.vector.tensor_copy(qraw, ps_q)
            sq_q = work.tile([128, TC], f16, tag="sq_q")
            nc.vector.tensor_mul(sq_q, qraw, qraw)
            yield
            # q stats: colsums of sq_q per head -> [2, TC]
            ss = psS.tile([128, TC], f32, tag="s", name="ss_q")
            nc.tensor.matmul(ss[0:2, :], lhsT=oc_sb, rhs=sq_q, start=True, stop=True)
            # q norm scalars: rr_q = (ss_q/64)^-0.5 via Ln+Exp, f16 for bc matmul
            lnq = work.tile([2, TC], f32, tag="lnq")
            nc.scalar.activation(lnq, ss[0:2, :], AF.Ln, scale=1.0 / 64.0)
            rrq = work.tile([2, TC], f16, tag="rrq")
            with nc.allow_low_precision(reason="1/rms ~1 fits fp16"):
                nc.scalar.activation(rrq, lnq, AF.Exp, scale=-0.5)
            yield
            bc = psS.tile([128, TC], f32, tag="s", name="bc_q")
            nc.tensor.matmul(bc, lhsT=obq_sb, rhs=rrq, start=True, stop=True)
            yield
            # RoPE q (w_q folded into cosq/sinq tables)
            rot_q = psA.tile([128, TC], f32, tag="kqv", name="rot_q")
            nc.tensor.matmul(rot_q, lhsT=p2_sb, rhs=qraw, start=True, stop=True)
            yield
            t1q = work.tile([128, TC], f16, tag="t1q")
            nc.vector.tensor_mul(t1q, qraw, cosq_sb[:, tsl])
            qshq = work.tile([128, TC], f16, tag="qshq")
            nc.vector.tensor_mul(qshq, rot_q, sinq_sb[:, tsl])
            t2q = work.tile([128, TC], f16, tag="t2q")
            nc.vector.tensor_add(t2q, t1q, qshq)
            nc.vector.tensor_mul(qT[p][:, tsl], t2q, bc)
            yield
            ps_k = psA.tile([128, TC], f32, tag="kqv", name="ps_k")
            for ci in range(8):
                nc.tensor.matmul(
                    ps_k,
                    lhsT=wk_sb[:, ci, p * 128 : (p + 1) * 128],
                    rhs=xt[:, ci, :],
                    start=(ci == 0),
                    stop=(ci == 7),
                )
                yield
            kraw = work.tile([128, TC], f16, tag="kraw")
            nc.vector.tensor_copy(kraw, ps_k)
            sq_k = work.tile([128, TC], f16, tag="sq_k")
            nc.vector.tensor_mul(sq_k, kraw, kraw)
            yield
            # k stats, transposed: ssT[s_part, st, h] via 4 tiny matmuls
            ss2 = psS.tile([128, TC], f32, tag="s", name="ss_k")
            for st in range(4):
                nc.tensor.matmul(
                    ss2[:, 2 * st : 2 * st + 2],
                    lhsT=sq_k[:, st * 128 : (st + 1) * 128],
                    rhs=oc_sb,
                    start=True,
                    stop=True,
                )
            # rkT = ss_k^-0.5 = exp(-0.5 ln(ss_k))  [0.125/sqrt(64) folded exactly]
            lnk = work.tile([128, 4, 2], f32, tag="lnk")
            nc.scalar.activation(
                lnk, ss2[:, 0:8].rearrange("p (a b) -> p a b", b=2), AF.Ln
            )
            nc.scalar.activation(
                rkT[:, tci * 4 : tci * 4 + 4, 2 * p : 2 * p + 2],
                lnk,
                AF.Exp,
                scale=-0.5,
            )
            yield
            # RoPE k (no norm here; k-norm applied via exp scale)
            rot_k = psA.tile([128, TC], f32, tag="kqv", name="rot_k")
            nc.tensor.matmul(rot_k, lhsT=p2_sb, rhs=kraw, start=True, stop=True)
            yield
            t1k = work.tile([128, TC], f16, tag="t1k")
            nc.vector.tensor_mul(t1k, kraw, cosk_sb[:, tsl])
            qshk = work.tile([128, TC], f16, tag="qshk")
            nc.vector.tensor_mul(qshk, rot_k, sink_sb[:, tsl])
            nc.vector.tensor_add(kT[p][:, tsl], t1k, qshk)
            yield

        def gen_a_v(tci, st):
            """v projection for subtile st of chunk tci."""
            xt = xts[tci]
            pv = psA.tile([128, TC], f32, tag="kqv", name="pv")
            for ci in range(8):
                nc.tensor.matmul(
                    pv,
                    lhsT=xt[:, ci, st * 128 : (st + 1) * 128],
                    rhs=wv_sb[:, ci, :],
                    start=(ci == 0),
                    stop=(ci == 7),
                )
                yield
            nc.vector.tensor_copy(
                v_sb[:, tci * 4 + st, :, 0:64],
                pv.rearrange("p (h d) -> p h d", h=HPC),
            )
            yield

        def gen_c_unit(tci, st_local, co):
            """Output projection for token tile st of chunk tci, half co."""
            st = tci * 4 + st_local
            po = psA.tile([128, TC], f32, tag="kqv", name="po")
            for p in range(4):
                nc.tensor.matmul(
                    po,
                    lhsT=yT[p][:, st * 128 : (st + 1) * 128],
                    rhs=wp_sb[:, p, co * 512 : (co + 1) * 512],
                    start=(p == 0),
                    stop=(p == 3),
                )
                yield
            ot = outp.tile([128, TC], f16, tag="o")
            with nc.allow_low_precision(reason="fp16 partials; host sums in fp32"):
                nc.vector.tensor_copy(ot, po)
            nc.sync.dma_start(
                outd[st * 128 : (st + 1) * 128, co * 512 : (co + 1) * 512], ot
            )
            yield

        def emit_b_head(tci, h, pop):
            """Attention for head h over this t-chunk; pop() drains filler."""
            p, hl = h // 2, h % 2
            hsl = slice(hl * 64, (hl + 1) * 64)
            n_s = 4 * (tci + 1)
            tbase = tci * TC
            tsl = slice(tbase, tbase + TC)

            def score(si):
                d = si * 128 - tbase if si >= 4 * tci else 0
                ps_s = psB.tile([128, TC], f32, tag="sc", name="ps_s")
                nc.tensor.matmul(
                    ps_s[:, d:TC],
                    lhsT=kT[p][hsl, si * 128 : (si + 1) * 128],
                    rhs=qT[p][hsl, tbase + d : tbase + TC],
                    start=True,
                    stop=True,
                )
                return ps_s, d

            ps_y = psY.tile([128, TC], f32, tag="y", name="ps_y")
            pend = [score(0)]
            for si in range(n_s):
                ps_s, d = pend.pop(0)
                at = attp.tile([128, TC], f16, tag="at")
                with nc.allow_low_precision(reason="softmax weights fp16"):
                    nc.scalar.activation(
                        at[:, d:TC],
                        ps_s[:, d:TC],
                        AF.Exp,
                        bias=bsh,
                        scale=rkT[:, si, h : h + 1],
                    )
                if si + 1 < n_s:
                    pend.append(score(si + 1))
                if si >= 4 * tci:  # diagonal subtile: triangular mask block
                    nc.vector.tensor_mul(
                        at[:, d : d + 128], at[:, d : d + 128], tri_sb
                    )
                nc.tensor.matmul(
                    ps_y[0:65, d:TC],
                    lhsT=v_sb[:, si, h, :],
                    rhs=at[:, d:TC],
                    start=(si == 0),
                    stop=(si == n_s - 1),
                )
                pop()
            # softmax denominator: row 64 of ps_y (shrunk by 2^-10, cancels)
            rec = work.tile([1, TC], f16, tag="rec")
            with nc.allow_low_precision(reason="1/denom bounded by shrink"):
                nc.vector.reciprocal(rec, ps_y[64:65, :])
            db_sb = work.tile([64, TC], f16, tag="db")
            nc.gpsimd.partition_broadcast(db_sb, rec)
            nc.vector.tensor_mul(yT[p][hsl, tsl], ps_y[0:64, :], db_sb)
            pop()

        def body():
            # prologue: x chunk 0 + projections for chunk 0
            dma_x(0, split=True)
            for p in range(4):
                for _ in gen_a_pair(0, p):
                    pass
            for st in range(4):
                for _ in gen_a_v(0, st):
                    pass
            dma_x(1)

            from itertools import chain

            N_PAIR_STEPS, N_V_STEPS, N_C_STEPS = 25, 9, 5
            for tci in range(NTC):
                if tci + 2 < NTC:
                    dma_x(tci + 2)
                gens = []
                total_fill = 0
                if tci + 1 < NTC:
                    for p in range(4):
                        gens.append(gen_a_pair(tci + 1, p))
                        total_fill += N_PAIR_STEPS
                    for st in range(4):
                        gens.append(gen_a_v(tci + 1, st))
                        total_fill += N_V_STEPS
                if tci > 0:
                    for st_local in range(4):
                        for co in range(2):
                            gens.append(gen_c_unit(tci - 1, st_local, co))
                            total_fill += N_C_STEPS
                fill_it = chain(*gens)

                steps_total = 8 * (4 * (tci + 1) + 1)
                state = {"step": 0, "emitted": 0}

                def pop():
                    state["step"] += 1
                    target = (state["step"] * total_fill + steps_total - 1) // steps_total
                    while state["emitted"] < target:
                        try:
                            next(fill_it)
                        except StopIteration:
                            state["emitted"] = total_fill
                            break
                        state["emitted"] += 1

                for h in range(HPC):
                    emit_b_head(tci, h, pop)
                # drain leftover filler
                for _ in fill_it:
                    pass

            # epilogue: output projection for last chunk
            for st_local in range(4):
                for co in range(2):
                    for _ in gen_c_unit(NTC - 1, st_local, co):
                        pass

        if loop_n is None:
            body()
        else:
            with tc.For_i(0, loop_n, 1):
                body()

    return nc


def _get_nc(loop_n=None):
    key = ("nc", loop_n)
    if key not in _STATE:
        nc = _build_nc(loop_n)
        nc.finalize()
        _STATE[key] = nc
    return _STATE[key]


def _rope_tables(w):
    """cosW/sinW [128, T] f16 with per-dim norm weight folded in."""
    inv_freq = 1.0 / (10000.0 ** (np.arange(0, D, 2, dtype=np.float64) / D))
    t_pos = np.arange(T, dtype=np.float64)
    freqs = t_pos[:, None] * inv_freq[None, :]  # [T, 32]
    f2 = np.concatenate([freqs, freqs], axis=-1)  # [T, 64]
    w = np.asarray(w, dtype=np.float64)[:, None]  # [64, 1]
    cosT = (w * np.cos(f2).T).astype(np.float16)  # [64, T]
    sinT = (w * np.sin(f2).T).astype(np.float16)
    cos2 = np.concatenate([cosT, cosT], axis=0)  # [128, T]
    sin2 = np.concatenate([sinT, sinT], axis=0)
    return np.ascontiguousarray(cos2), np.ascontiguousarray(sin2)


def _prep_inputs(x, W_kqv, W_proj, q_norm_w, k_norm_w):
    x = np.asarray(x, dtype=np.float32)
    W_kqv = np.asarray(W_kqv, dtype=np.float32)
    W_proj = np.asarray(W_proj, dtype=np.float32)
    q_norm_w = np.asarray(q_norm_w, dtype=np.float32)
    k_norm_w = np.asarray(k_norm_w, dtype=np.float32)

    cosq, sinq = _rope_tables(q_norm_w)
    cosk, sink = _rope_tables(k_norm_w)

    # triangular mask for the diagonal 128-block: keep t >= s
    si = np.arange(128)[:, None]
    cj = np.arange(128)[None, :]
    tri = (cj >= si).astype(np.float16)

    # oc: colsum selectors per head half
    oc = np.zeros((128, 2), dtype=np.float16)
    oc[0:64, 0] = 1.0
    oc[64:128, 1] = 1.0

    # obq: expand rr rows to 64-partition halves (pure ones; w in tables)
    obq = np.zeros((2, 128), dtype=np.float16)
    obq[0, 0:64] = 1.0
    obq[1, 64:128] = 1.0

    # signed rotate-half permutation (per 64-dim head, stacked twice)
    P = np.zeros((64, 64), dtype=np.float16)
    for i in range(32):
        P[i, i + 32] = -1.0
        P[i + 32, i] = 1.0
    P2 = np.zeros((128, 128), dtype=np.float16)
    P2[0:64, 0:64] = P
    P2[64:128, 64:128] = P
    p2T = np.ascontiguousarray(P2.T)

    def wt_kqv(rows):
        # rows: [512, 1024] -> lhsT layout [128, 8, 512] fp16
        wT = rows.T.astype(np.float16)  # [1024, 512]
        return np.ascontiguousarray(wT.reshape(8, 128, 512).transpose(1, 0, 2))

    Wk, Wq, Wv = W_kqv[0:C], W_kqv[C : 2 * C], W_kqv[2 * C : 3 * C]

    in_maps = []
    for c in range(N_CORES):
        b, g = c // 2, c % 2
        rs = slice(512 * g, 512 * (g + 1))
        xTb = x[b].T.astype(np.float16)  # [C, T]
        xTr = np.ascontiguousarray(
            xTb.reshape(8, 128, NTC, TC).transpose(2, 1, 0, 3)
        )  # [NTC, 128, 8, TC]
        wp = W_proj[:, rs].T.astype(np.float16)  # [512, 1024]
        wpr = np.ascontiguousarray(wp.reshape(4, 128, 1024).transpose(1, 0, 2))
        in_maps.append(
            {
                "xT": xTr,
                "wqT": wt_kqv(Wq[rs]),
                "wkT": wt_kqv(Wk[rs]),
                "wvT": wt_kqv(Wv[rs]),
                "wpT": wpr,
                "cosq": cosq,
                "sinq": sinq,
                "cosk": cosk,
                "sink": sink,
                "trid": tri,
                "p2d": p2T,
                "ocd": oc,
                "obqd": obq,
            }
        )
    return in_maps


def _get_runner(loop_n=None):
    """Build (once) a cached jitted SPMD runner mirroring
    bass2jax.run_bass_via_pjrt, so repeated calls reuse the compiled NEFF."""
    key = ("runner", loop_n)
    if key in _STATE:
        return _STATE[key]

    import jax
    import concourse.mybir as mybir
    from concourse import bass2jax
    from concourse.bass2jax import _bass_exec_p, partition_id_tensor
    from jax.experimental.shard_map import shard_map
    from jax.sharding import Mesh, NamedSharding, PartitionSpec

    bass2jax.install_neuronx_cc_hook()
    nc = _get_nc(loop_n)

    partition_name = nc.partition_id_tensor.name if nc.partition_id_tensor else None
    in_names, out_names, out_avals, zero_outs = [], [], [], []
    for alloc in nc.m.functions[0].allocations:
        if not isinstance(alloc, mybir.MemoryLocationSet):
            continue
        name = alloc.memorylocations[0].name
        if alloc.kind == "ExternalInput":
            if name != partition_name:
                in_names.append(name)
        elif alloc.kind == "ExternalOutput":
            shape = tuple(alloc.tensor_shape)
            dtype = mybir.dt.np(alloc.dtype)
            out_names.append(name)
            out_avals.append(jax.core.ShapedArray(shape, dtype))
            zero_outs.append(np.zeros(shape, dtype))
    n_params = len(in_names)
    all_names = in_names + out_names
    if partition_name is not None:
        all_names.append(partition_name)

    def _body(*args):
        operands = list(args)
        if partition_name is not None:
            operands.append(partition_id_tensor())
        outs = _bass_exec_p.bind(
            *operands,
            out_avals=tuple(out_avals),
            in_names=tuple(all_names),
            out_names=tuple(out_names),
            lowering_input_output_aliases=(),
            sim_require_finite=True,
            sim_require_nnan=True,
            nc=nc,
        )
        return tuple(outs)

    devices = jax.devices()[:N_CORES]
    mesh = Mesh(np.asarray(devices), ("core",))
    spec = PartitionSpec("core")
    n_outs = len(out_names)
    sharded = jax.jit(
        shard_map(
            _body,
            mesh=mesh,
            in_specs=(spec,) * (n_params + n_outs),
            out_specs=(spec,) * n_outs,
            check_rep=False,
        ),
        keep_unused=True,
    )
    sharding = NamedSharding(mesh, spec)
    zeros_dev = [
        jax.device_put(
            np.zeros((N_CORES * z.shape[0], *z.shape[1:]), z.dtype), sharding
        )
        for z in zero_outs
    ]
    runner = {
        "sharded": sharded,
        "in_names": in_names,
        "out_names": out_names,
        "out_avals": out_avals,
        "zeros_dev": zeros_dev,
        "sharding": sharding,
    }
    _STATE[key] = runner
    return runner


def _concat_inputs(in_maps, runner):
    return [
        np.concatenate([np.asarray(in_maps[c][n]) for c in range(N_CORES)], axis=0)
        for n in runner["in_names"]
    ]


def _execute(in_maps):
    """Returns list (per core) of {out_name: np.ndarray}."""
    runner = _get_runner()
    concat_in = _concat_inputs(in_maps, runner)
    out_arrs = runner["sharded"](*concat_in, *runner["zeros_dev"])
    return [
        {
            n: np.asarray(out_arrs[i]).reshape(
                N_CORES, *runner["out_avals"][i].shape
            )[c]
            for i, n in enumerate(runner["out_names"])
        }
        for c in range(N_CORES)
    ]


def _wall(runner, in_maps, iters):
    import time
    import jax

    concat_in = [
        jax.device_put(a, runner["sharding"])
        for a in _concat_inputs(in_maps, runner)
    ]
    args = (*concat_in, *runner["zeros_dev"])
    jax.block_until_ready(runner["sharded"](*args))  # warmup
    times = []
    for _ in range(iters):
        t0 = time.perf_counter()
        jax.block_until_ready(runner["sharded"](*args))
        times.append(time.perf_counter() - t0)
    times.sort()
    return times


def _timed(in_maps, iters=20, n_lo=1, n_hi=33):
    """Per-pass HW time via two device-side repeat counts: the dispatch/tunnel
    overhead cancels in the difference."""
    r_lo = _get_runner(None if n_lo == 1 else n_lo)
    r_hi = _get_runner(n_hi)
    t_lo = _wall(r_lo, in_maps, iters)
    t_hi = _wall(r_hi, in_maps, iters)
    k = max(3, iters // 4)
    lo = sum(t_lo[:k]) / k
    hi = sum(t_hi[:k]) / k
    per_pass = (hi - lo) / (n_hi - n_lo)
    return per_pass, lo, hi


def kernel(**inputs):
    in_maps = _prep_inputs(**inputs)
    res = _execute(in_maps)
    out = np.zeros((B, T, C), dtype=np.float32)
    for c in range(N_CORES):
        out[c // 2] += res[c]["out"].astype(np.float32)
    return out


# revision 7
# speedup vs baseline: 1.0856x; 1.0533x over previous
"""Multi-head causal attention (RoPE + per-head RMSNorm) on 8 TRN2 NeuronCores.

v2: activation-table churn removed (Ln/Exp only on ACT engine), k-norm folded
into exp's per-partition scale AP, denominator via fp32 reciprocal + exp bias
shrink, partial-column diagonal tiles, software-pipelined emission that keeps
the in-order PE queue fed (score(si+1) before AV(si), next-chunk projections
and prev-chunk output projection interleaved at si granularity).

Sharding: core c -> batch b = c//2, head group g = c%2 (heads 8g..8g+8).
Each core computes a partial out[b] over its 8 heads' channels; host sums the
two partials per batch.
"""

import sys

import numpy as np

sys.path.insert(0, "/opt/trn_rl_repo")

B, T, C, H, D = 4, 2048, 1024, 16, 64
N_CORES = 8
HPC = H // 2  # heads per core: 8
TC = 512  # t-chunk (matmul free dim)
NTC = T // TC  # 4
NST = T // 128  # 16 s/t subtiles
LOG_SHRINK = -6.931471805599453  # ln(2^-10): at' shrink so unnormalized y fits fp16

_STATE: dict = {}

# within each 32-partition quadrant: swap adjacent pairs (2j <-> 2j+1)
_SWAP_MASK = [j + 1 if j % 2 == 0 else j - 1 for j in range(32)]


def _force_combined_exp_ln_table():
    """Make the act-table chooser pick natural_log_exp_and_others for both
    Exp and Ln (greedy first-match otherwise alternates between the exp-only
    and ln-only tables, costing a 1283ns table reload per switch). Table list
    positions are preserved; only the claimed function sets shrink."""
    if _STATE.get("tables_patched"):
        return
    import concourse.bacc as bacc
    import concourse.mybir as mybir

    orig = bacc.get_activation_tables
    keep = "natural_log_exp_and_others"
    drop = {mybir.ActivationFunctionType.Exp, mybir.ActivationFunctionType.Ln}

    def patched(arch):
        tabs = orig(arch)
        return {
            name: (funcs if name == keep else funcs - drop)
            for name, funcs in tabs.items()
        }

    bacc.get_activation_tables = patched
    _STATE["tables_patched"] = True


def _build_nc(loop_n=None):
    _force_combined_exp_ln_table()
    import concourse.mybir as mybir
    from concourse import bacc
    from concourse.tile import TileContext
    from contextlib import ExitStack

    f16 = mybir.dt.float16
    f32 = mybir.dt.float32
    AF = mybir.ActivationFunctionType

    nc = bacc.Bacc(
        "TRN2",
        target_bir_lowering=False,
        debug=False,
        num_devices=N_CORES,
    )

    xT = nc.dram_tensor("xT", [NTC, 128, 8, TC], f16, kind="ExternalInput")
    wqT = nc.dram_tensor("wqT", [128, 8, 512], f16, kind="ExternalInput")
    wkT = nc.dram_tensor("wkT", [128, 8, 512], f16, kind="ExternalInput")
    wvT = nc.dram_tensor("wvT", [128, 8, 512], f16, kind="ExternalInput")
    wpT = nc.dram_tensor("wpT", [128, 4, 1024], f16, kind="ExternalInput")
    cosq = nc.dram_tensor("cosq", [128, T], f16, kind="ExternalInput")
    sinq = nc.dram_tensor("sinq", [128, T], f16, kind="ExternalInput")
    cosk = nc.dram_tensor("cosk", [128, T], f16, kind="ExternalInput")
    sink = nc.dram_tensor("sink", [128, T], f16, kind="ExternalInput")
    trid = nc.dram_tensor("trid", [128, 128], f16, kind="ExternalInput")
    ocd = nc.dram_tensor("ocd", [128, 2], f16, kind="ExternalInput")
    obqd = nc.dram_tensor("obqd", [2, 128], f16, kind="ExternalInput")
    outd = nc.dram_tensor("out", [T, C], f16, kind="ExternalOutput")

    with TileContext(nc) as tc, ExitStack() as ctx:
        const = ctx.enter_context(tc.tile_pool(name="const", bufs=1))
        xpool = ctx.enter_context(tc.tile_pool(name="xp", bufs=2))
        persist = ctx.enter_context(tc.tile_pool(name="persist", bufs=1))
        work = ctx.enter_context(tc.tile_pool(name="work", bufs=4))
        attp = ctx.enter_context(tc.tile_pool(name="attp", bufs=6))
        outp = ctx.enter_context(tc.tile_pool(name="outp", bufs=2))
        psA = ctx.enter_context(tc.tile_pool(name="psA", bufs=2, space="PSUM"))
        psB = ctx.enter_context(tc.tile_pool(name="psB", bufs=2, space="PSUM"))
        psY = ctx.enter_context(tc.tile_pool(name="psY", bufs=2, space="PSUM"))
        psS = ctx.enter_context(tc.tile_pool(name="psS", bufs=2, space="PSUM"))

        # ---- constants (DMA'd in first-use order) ----
        wq_sb = const.tile([128, 8, 512], f16, tag="wq")
        wk_sb = const.tile([128, 8, 512], f16, tag="wk")
        for ci in range(8):  # split so the first matmuls start at 1/8 loaded
            nc.sync.dma_start(wq_sb[:, ci, :], wqT[:, ci, :])
            nc.sync.dma_start(wk_sb[:, ci, :], wkT[:, ci, :])
        cosq_sb = const.tile([128, T], f16, tag="cosq")
        nc.sync.dma_start(cosq_sb, cosq[:, :])
        sinq_sb = const.tile([128, T], f16, tag="sinq")
        nc.sync.dma_start(sinq_sb, sinq[:, :])
        cosk_sb = const.tile([128, T], f16, tag="cosk")
        nc.sync.dma_start(cosk_sb, cosk[:, :])
        sink_sb = const.tile([128, T], f16, tag="sink")
        nc.sync.dma_start(sink_sb, sink[:, :])
        oc_sb = const.tile([128, 2], f16, tag="oc")
        nc.sync.dma_start(oc_sb, ocd[:, :])
        obq_sb = const.tile([2, 128], f16, tag="obq")
        nc.sync.dma_start(obq_sb, obqd[:, :])
        wv_sb = const.tile([128, 8, 512], f16, tag="wv")
        nc.sync.dma_start(wv_sb, wvT[:, :, :])
        tri_sb = const.tile([128, 128], f16, tag="tri")
        nc.sync.dma_start(tri_sb, trid[:, :])
        wp_sb = const.tile([128, 4, 1024], f16, tag="wp")
        nc.sync.dma_start(wp_sb, wpT[:, :, :])
        bsh = const.tile([128, 1], f32, tag="bsh")
        nc.vector.memset(bsh, LOG_SHRINK)

        # ---- persistent activations ----
        qT = [
            persist.tile([128, T], f16, tag=f"qT{p}", name=f"qT{p}")
            for p in range(4)
        ]
        kT = [
            persist.tile([128, T], f16, tag=f"kT{p}", name=f"kT{p}")
            for p in range(4)
        ]
        yT = [
            persist.tile([128, T], f16, tag=f"yT{p}", name=f"yT{p}")
            for p in range(4)
        ]
        v_sb = persist.tile([128, NST, HPC, 65], f16, tag="v")
        nc.vector.memset(v_sb[:, :, :, 64:65], 1.0)
        # rkT[s_part, si, h] = ss_k^-1/2 (= 0.125/rms_k); f32 for exp scale AP
        rkT = persist.tile([128, NST, HPC], f32, tag="rkT")

        xts = {}

        def dma_x(tci, split=False):
            xt = xpool.tile([128, 8, TC], f16, tag="x", name=f"xt{tci}")
            if split:
                for ci in range(8):
                    nc.sync.dma_start(xt[:, ci, :], xT[tci, :, ci, :])
            else:
                nc.sync.dma_start(xt, xT[tci])
            xts[tci] = xt

        def gen_a_pair(tci, p):
            """Projection + RoPE + stats for head pair p of chunk tci.
            Yields after each atomic step (~1 PE matmul of work)."""
            xt = xts[tci]
            tsl = slice(tci * TC, (tci + 1) * TC)
            ps_q = psA.tile([128, TC], f32, tag="kqv", name="ps_q")
            for ci in range(8):
                nc.tensor.matmul(
                    ps_q,
                    lhsT=wq_sb[:, ci, p * 128 : (p + 1) * 128],
                    rhs=xt[:, ci, :],
                    start=(ci == 0),
                    stop=(ci == 7),
                )
                yield
            qraw = work.tile([128, TC], f16, tag="qraw")
            nc.vector.tensor_copy(qraw, ps_q)
            sq_q = work.tile([128, TC], f16, tag="sq_q")
            nc.vector.tensor_mul(sq_q, qraw, qraw)
            yield
            # q stats: colsums of sq_q per head -> [2, TC]
            ss = psS.tile([128, TC], f32, tag="s", name="ss_q")
            nc.tensor.matmul(ss[0:2, :], lhsT=oc_sb, rhs=sq_q, start=True, stop=True)
            # q norm scalars: rr_q = (ss_q/64)^-0.5 via Ln+Exp, f16 for bc matmul
            lnq = work.tile([2, TC], f32, tag="lnq")
            nc.scalar.activation(lnq, ss[0:2, :], AF.Ln, scale=1.0 / 64.0)
            rrq = work.tile([2, TC], f16, tag="rrq")
            with nc.allow_low_precision(reason="1/rms ~1 fits fp16"):
                nc.scalar.activation(rrq, lnq, AF.Exp, scale=-0.5)
            yield
            bc = psS.tile([128, TC], f32, tag="s", name="bc_q")
            nc.tensor.matmul(bc, lhsT=obq_sb, rhs=rrq, start=True, stop=True)
            yield
            # RoPE q: interleaved d-layout makes rotate-half a partition swap
            # (signs and w_q folded into the sinq/cosq tables)
            t1q = work.tile([128, TC], f16, tag="t1q")
            nc.vector.tensor_mul(t1q, qraw, cosq_sb[:, tsl])
            sh_q = work.tile([128, TC], f16, tag="sh_q")
            nc.vector.stream_shuffle(sh_q, qraw, _SWAP_MASK)
            qshq = work.tile([128, TC], f16, tag="qshq")
            nc.vector.tensor_mul(qshq, sh_q, sinq_sb[:, tsl])
            t2q = work.tile([128, TC], f16, tag="t2q")
            nc.vector.tensor_add(t2q, t1q, qshq)
            nc.vector.tensor_mul(qT[p][:, tsl], t2q, bc)
            yield
            ps_k = psA.tile([128, TC], f32, tag="kqv", name="ps_k")
            for ci in range(8):
                nc.tensor.matmul(
                    ps_k,
                    lhsT=wk_sb[:, ci, p * 128 : (p + 1) * 128],
                    rhs=xt[:, ci, :],
                    start=(ci == 0),
                    stop=(ci == 7),
                )
                yield
            kraw = work.tile([128, TC], f16, tag="kraw")
            nc.vector.tensor_copy(kraw, ps_k)
            sq_k = work.tile([128, TC], f16, tag="sq_k")
            nc.vector.tensor_mul(sq_k, kraw, kraw)
            yield
            # k stats, transposed: ssT[s_part, st, h] via 4 tiny matmuls
            ss2 = psS.tile([128, TC], f32, tag="s", name="ss_k")
            for st in range(4):
                nc.tensor.matmul(
                    ss2[:, 2 * st : 2 * st + 2],
                    lhsT=sq_k[:, st * 128 : (st + 1) * 128],
                    rhs=oc_sb,
                    start=True,
                    stop=True,
                )
            # rkT = ss_k^-0.5 = exp(-0.5 ln(ss_k))  [0.125/sqrt(64) folded exactly]
            lnk = work.tile([128, 4, 2], f32, tag="lnk")
            nc.scalar.activation(
                lnk, ss2[:, 0:8].rearrange("p (a b) -> p a b", b=2), AF.Ln
            )
            nc.scalar.activation(
                rkT[:, tci * 4 : tci * 4 + 4, 2 * p : 2 * p + 2],
                lnk,
                AF.Exp,
                scale=-0.5,
            )
            yield
            # RoPE k (no norm here; k-norm applied via exp scale)
            t1k = work.tile([128, TC], f16, tag="t1k")
            nc.vector.tensor_mul(t1k, kraw, cosk_sb[:, tsl])
            sh_k = work.tile([128, TC], f16, tag="sh_k")
            nc.vector.stream_shuffle(sh_k, kraw, _SWAP_MASK)
            qshk = work.tile([128, TC], f16, tag="qshk")
            nc.vector.tensor_mul(qshk, sh_k, sink_sb[:, tsl])
            nc.vector.tensor_add(kT[p][:, tsl], t1k, qshk)
            yield

        def gen_a_v(tci, st):
            """v projection for subtile st of chunk tci."""
            xt = xts[tci]
            pv = psA.tile([128, TC], f32, tag="kqv", name="pv")
            for ci in range(8):
                nc.tensor.matmul(
                    pv,
                    lhsT=xt[:, ci, st * 128 : (st + 1) * 128],
                    rhs=wv_sb[:, ci, :],
                    start=(ci == 0),
                    stop=(ci == 7),
                )
                yield
            nc.vector.tensor_copy(
                v_sb[:, tci * 4 + st, :, 0:64],
                pv.rearrange("p (h d) -> p h d", h=HPC),
            )
            yield

        def gen_c_unit(tci, st_local, co):
            """Output projection for token tile st of chunk tci, half co."""
            st = tci * 4 + st_local
            po = psA.tile([128, TC], f32, tag="kqv", name="po")
            for p in range(4):
                nc.tensor.matmul(
                    po,
                    lhsT=yT[p][:, st * 128 : (st + 1) * 128],
                    rhs=wp_sb[:, p, co * 512 : (co + 1) * 512],
                    start=(p == 0),
                    stop=(p == 3),
                )
                yield
            ot = outp.tile([128, TC], f16, tag="o")
            with nc.allow_low_precision(reason="fp16 partials; host sums in fp32"):
                nc.vector.tensor_copy(ot, po)
            nc.sync.dma_start(
                outd[st * 128 : (st + 1) * 128, co * 512 : (co + 1) * 512], ot
            )
            yield

        def emit_b_head(tci, h, pop):
            """Attention for head h over this t-chunk; pop() drains filler."""
            p, hl = h // 2, h % 2
            hsl = slice(hl * 64, (hl + 1) * 64)
            n_s = 4 * (tci + 1)
            tbase = tci * TC
            tsl = slice(tbase, tbase + TC)

            def score(si):
                d = si * 128 - tbase if si >= 4 * tci else 0
                ps_s = psB.tile([128, TC], f32, tag="sc", name="ps_s")
                nc.tensor.matmul(
                    ps_s[:, d:TC],
                    lhsT=kT[p][hsl, si * 128 : (si + 1) * 128],
                    rhs=qT[p][hsl, tbase + d : tbase + TC],
                    start=True,
                    stop=True,
                )
                return ps_s, d

            ps_y = psY.tile([128, TC], f32, tag="y", name="ps_y")
            pend = [score(0)]
            for si in range(n_s):
                ps_s, d = pend.pop(0)
                at = attp.tile([128, TC], f16, tag="at")
                with nc.allow_low_precision(reason="softmax weights fp16"):
                    nc.scalar.activation(
                        at[:, d:TC],
                        ps_s[:, d:TC],
                        AF.Exp,
                        bias=bsh,
                        scale=rkT[:, si, h : h + 1],
                    )
                if si + 1 < n_s:
                    pend.append(score(si + 1))
                if si >= 4 * tci:  # diagonal subtile: triangular mask block
                    nc.vector.tensor_mul(
                        at[:, d : d + 128], at[:, d : d + 128], tri_sb
                    )
                nc.tensor.matmul(
                    ps_y[0:65, d:TC],
                    lhsT=v_sb[:, si, h, :],
                    rhs=at[:, d:TC],
                    start=(si == 0),
                    stop=(si == n_s - 1),
                )
                pop()
            # softmax denominator: row 64 of ps_y (shrunk by 2^-10, cancels)
            rec = work.tile([1, TC], f16, tag="rec")
            with nc.allow_low_precision(reason="1/denom bounded by shrink"):
                nc.vector.reciprocal(rec, ps_y[64:65, :])
            db_sb = work.tile([64, TC], f16, tag="db")
            nc.gpsimd.partition_broadcast(db_sb, rec)
            nc.vector.tensor_mul(yT[p][hsl, tsl], ps_y[0:64, :], db_sb)
            pop()

        def body():
            # prologue: x chunk 0 + projections for chunk 0
            dma_x(0, split=True)
            for p in range(4):
                for _ in gen_a_pair(0, p):
                    pass
            for st in range(4):
                for _ in gen_a_v(0, st):
                    pass
            dma_x(1)

            from itertools import chain

            N_PAIR_STEPS, N_V_STEPS, N_C_STEPS = 23, 9, 5
            for tci in range(NTC):
                if tci + 2 < NTC:
                    dma_x(tci + 2)
                gens = []
                total_fill = 0
                if tci + 1 < NTC:
                    for p in range(4):
                        gens.append(gen_a_pair(tci + 1, p))
                        total_fill += N_PAIR_STEPS
                    for st in range(4):
                        gens.append(gen_a_v(tci + 1, st))
                        total_fill += N_V_STEPS
                if tci > 0:
                    for st_local in range(4):
                        for co in range(2):
                            gens.append(gen_c_unit(tci - 1, st_local, co))
                            total_fill += N_C_STEPS
                fill_it = chain(*gens)

                steps_total = 8 * (4 * (tci + 1) + 1)
                state = {"step": 0, "emitted": 0}

                def pop():
                    state["step"] += 1
                    target = (state["step"] * total_fill + steps_total - 1) // steps_total
                    while state["emitted"] < target:
                        try:
                            next(fill_it)
                        except StopIteration:
                            state["emitted"] = total_fill
                            break
                        state["emitted"] += 1

                for h in range(HPC):
                    emit_b_head(tci, h, pop)
                # drain leftover filler
                for _ in fill_it:
                    pass

            # epilogue: output projection for last chunk
            for st_local in range(4):
                for co in range(2):
                    for _ in gen_c_unit(NTC - 1, st_local, co):
                        pass

        if loop_n is None:
            body()
        else:
            with tc.For_i(0, loop_n, 1):
                body()

    return nc


def _get_nc(loop_n=None):
    key = ("nc", loop_n)
    if key not in _STATE:
        nc = _build_nc(loop_n)
        nc.finalize()
        _STATE[key] = nc
    return _STATE[key]


def _d_order():
    """Interleaved head-dim order: position 2j holds dim j, 2j+1 holds dim j+32,
    so rotate-half is a swap of adjacent partitions (stream_shuffle)."""
    order = np.empty(D, dtype=np.int64)
    order[0::2] = np.arange(32)
    order[1::2] = np.arange(32) + 32
    return order


def _rope_tables(w):
    """cosW/sinW [128, T] f16 in interleaved d-order, norm weight and the
    rotate-half signs folded in."""
    inv_freq = 1.0 / (10000.0 ** (np.arange(0, D, 2, dtype=np.float64) / D))
    t_pos = np.arange(T, dtype=np.float64)
    freqs = t_pos[:, None] * inv_freq[None, :]  # [T, 32]
    f2 = np.concatenate([freqs, freqs], axis=-1)  # [T, 64]
    w = np.asarray(w, dtype=np.float64)  # [64]
    order = _d_order()
    sign = np.where(order < 32, -1.0, 1.0)  # rot[d] = -x[d+32] (d<32), +x[d-32]
    cosB = w[:, None] * np.cos(f2).T  # [64, T], plain d order
    sinB = w[:, None] * np.sin(f2).T
    cosT = cosB[order].astype(np.float16)
    sinT = (sign[:, None] * sinB[order]).astype(np.float16)
    cos2 = np.concatenate([cosT, cosT], axis=0)  # [128, T]
    sin2 = np.concatenate([sinT, sinT], axis=0)
    return np.ascontiguousarray(cos2), np.ascontiguousarray(sin2)


def _prep_inputs(x, W_kqv, W_proj, q_norm_w, k_norm_w):
    x = np.asarray(x, dtype=np.float32)
    W_kqv = np.asarray(W_kqv, dtype=np.float32)
    W_proj = np.asarray(W_proj, dtype=np.float32)
    q_norm_w = np.asarray(q_norm_w, dtype=np.float32)
    k_norm_w = np.asarray(k_norm_w, dtype=np.float32)

    cosq, sinq = _rope_tables(q_norm_w)
    cosk, sink = _rope_tables(k_norm_w)

    # triangular mask for the diagonal 128-block: keep t >= s
    si = np.arange(128)[:, None]
    cj = np.arange(128)[None, :]
    tri = (cj >= si).astype(np.float16)

    # oc: colsum selectors per head half
    oc = np.zeros((128, 2), dtype=np.float16)
    oc[0:64, 0] = 1.0
    oc[64:128, 1] = 1.0

    # obq: expand rr rows to 64-partition halves (pure ones; w in tables)
    obq = np.zeros((2, 128), dtype=np.float16)
    obq[0, 0:64] = 1.0
    obq[1, 64:128] = 1.0

    order = _d_order()

    def wt_kqv(rows, perm=False):
        # rows: [512, 1024] -> lhsT layout [128, 8, 512] fp16
        if perm:  # interleave d within each head (q/k only)
            rows = rows.reshape(8, D, C)[:, order, :].reshape(512, C)
        wT = rows.T.astype(np.float16)  # [1024, 512]
        return np.ascontiguousarray(wT.reshape(8, 128, 512).transpose(1, 0, 2))

    Wk, Wq, Wv = W_kqv[0:C], W_kqv[C : 2 * C], W_kqv[2 * C : 3 * C]

    in_maps = []
    for c in range(N_CORES):
        b, g = c // 2, c % 2
        rs = slice(512 * g, 512 * (g + 1))
        xTb = x[b].T.astype(np.float16)  # [C, T]
        xTr = np.ascontiguousarray(
            xTb.reshape(8, 128, NTC, TC).transpose(2, 1, 0, 3)
        )  # [NTC, 128, 8, TC]
        wp = W_proj[:, rs].T.astype(np.float16)  # [512, 1024]
        wpr = np.ascontiguousarray(wp.reshape(4, 128, 1024).transpose(1, 0, 2))
        in_maps.append(
            {
                "xT": xTr,
                "wqT": wt_kqv(Wq[rs], perm=True),
                "wkT": wt_kqv(Wk[rs], perm=True),
                "wvT": wt_kqv(Wv[rs]),
                "wpT": wpr,
                "cosq": cosq,
                "sinq": sinq,
                "cosk": cosk,
                "sink": sink,
                "trid": tri,
                "ocd": oc,
                "obqd": obq,
            }
        )
    return in_maps


def _get_runner(loop_n=None):
    """Build (once) a cached jitted SPMD runner mirroring
    bass2jax.run_bass_via_pjrt, so repeated calls reuse the compiled NEFF."""
    key = ("runner", loop_n)
    if key in _STATE:
        return _STATE[key]

    import jax
    import concourse.mybir as mybir
    from concourse import bass2jax
    from concourse.bass2jax import _bass_exec_p, partition_id_tensor
    from jax.experimental.shard_map import shard_map
    from jax.sharding import Mesh, NamedSharding, PartitionSpec

    bass2jax.install_neuronx_cc_hook()
    nc = _get_nc(loop_n)

    partition_name = nc.partition_id_tensor.name if nc.partition_id_tensor else None
    in_names, out_names, out_avals, zero_outs = [], [], [], []
    for alloc in nc.m.functions[0].allocations:
        if not isinstance(alloc, mybir.MemoryLocationSet):
            continue
        name = alloc.memorylocations[0].name
        if alloc.kind == "ExternalInput":
            if name != partition_name:
                in_names.append(name)
        elif alloc.kind == "ExternalOutput":
            shape = tuple(alloc.tensor_shape)
            dtype = mybir.dt.np(alloc.dtype)
            out_names.append(name)
            out_avals.append(jax.core.ShapedArray(shape, dtype))
            zero_outs.append(np.zeros(shape, dtype))
    n_params = len(in_names)
    all_names = in_names + out_names
    if partition_name is not None:
        all_names.append(partition_name)

    def _body(*args):
        operands = list(args)
        if partition_name is not None:
            operands.append(partition_id_tensor())
        outs = _bass_exec_p.bind(
            *operands,
            out_avals=tuple(out_avals),
            in_names=tuple(all_names),
            out_names=tuple(out_names),
            lowering_input_output_aliases=(),
            sim_require_finite=True,
            sim_require_nnan=True,
            nc=nc,
        )
        return tuple(outs)

    devices = jax.devices()[:N_CORES]
    mesh = Mesh(np.asarray(devices), ("core",))
    spec = PartitionSpec("core")
    n_outs = len(out_names)
    sharded = jax.jit(
        shard_map(
            _body,
            mesh=mesh,
            in_specs=(spec,) * (n_params + n_outs),
            out_specs=(spec,) * n_outs,
            check_rep=False,
        ),
        keep_unused=True,
    )
    sharding = NamedSharding(mesh, spec)
    zeros_dev = [
        jax.device_put(
            np.zeros((N_CORES * z.shape[0], *z.shape[1:]), z.dtype), sharding
        )
        for z in zero_outs
    ]
    runner = {
        "sharded": sharded,
        "in_names": in_names,
        "out_names": out_names,
        "out_avals": out_avals,
        "zeros_dev": zeros_dev,
        "sharding": sharding,
    }
    _STATE[key] = runner
    return runner


def _concat_inputs(in_maps, runner):
    return [
        np.concatenate([np.asarray(in_maps[c][n]) for c in range(N_CORES)], axis=0)
        for n in runner["in_names"]
    ]


def _execute(in_maps):
    """Returns list (per core) of {out_name: np.ndarray}."""
    runner = _get_runner()
    concat_in = _concat_inputs(in_maps, runner)
    out_arrs = runner["sharded"](*concat_in, *runner["zeros_dev"])
    return [
        {
            n: np.asarray(out_arrs[i]).reshape(
                N_CORES, *runner["out_avals"][i].shape
            )[c]
            for i, n in enumerate(runner["out_names"])
        }
        for c in range(N_CORES)
    ]


def _wall(runner, in_maps, iters):
    import time
    import jax

    concat_in = [
        jax.device_put(a, runner["sharding"])
        for a in _concat_inputs(in_maps, runner)
    ]
    args = (*concat_in, *runner["zeros_dev"])
    jax.block_until_ready(runner["sharded"](*args))  # warmup
    times = []
    for _ in range(iters):
        t0 = time.perf_counter()
        jax.block_until_ready(runner["sharded"](*args))
        times.append(time.perf_counter() - t0)
    times.sort()
    return times


def _timed(in_maps, iters=20, n_lo=1, n_hi=33):
    """Per-pass HW time via two device-side repeat counts: the dispatch/tunnel
    overhead cancels in the difference."""
    r_lo = _get_runner(None if n_lo == 1 else n_lo)
    r_hi = _get_runner(n_hi)
    t_lo = _wall(r_lo, in_maps, iters)
    t_hi = _wall(r_hi, in_maps, iters)
    k = max(3, iters // 4)
    lo = sum(t_lo[:k]) / k
    hi = sum(t_hi[:k]) / k
    per_pass = (hi - lo) / (n_hi - n_lo)
    return per_pass, lo, hi


def kernel(**inputs):
    in_maps = _prep_inputs(**inputs)
    res = _execute(in_maps)
    out = np.zeros((B, T, C), dtype=np.float32)
    for c in range(N_CORES):
        out[c // 2] += res[c]["out"].astype(np.float32)
    return out


# revision 8
# speedup vs baseline: 1.0888x; 1.0030x over previous
"""Multi-head causal attention (RoPE + per-head RMSNorm) on 8 TRN2 NeuronCores.

v2: activation-table churn removed (Ln/Exp only on ACT engine), k-norm folded
into exp's per-partition scale AP, denominator via fp32 reciprocal + exp bias
shrink, partial-column diagonal tiles, software-pipelined emission that keeps
the in-order PE queue fed (score(si+1) before AV(si), next-chunk projections
and prev-chunk output projection interleaved at si granularity).

Sharding: core c -> batch b = c//2, head group g = c%2 (heads 8g..8g+8).
Each core computes a partial out[b] over its 8 heads' channels; host sums the
two partials per batch.
"""

import sys

import numpy as np

sys.path.insert(0, "/opt/trn_rl_repo")

B, T, C, H, D = 4, 2048, 1024, 16, 64
N_CORES = 8
HPC = H // 2  # heads per core: 8
TC = 512  # t-chunk (matmul free dim)
NTC = T // TC  # 4
NST = T // 128  # 16 s/t subtiles
LOG_SHRINK = -6.931471805599453  # ln(2^-10): at' shrink so unnormalized y fits fp16

_STATE: dict = {}

# within each 32-partition quadrant: swap adjacent pairs (2j <-> 2j+1)
_SWAP_MASK = [j + 1 if j % 2 == 0 else j - 1 for j in range(32)]


def _force_combined_exp_ln_table():
    """Make the act-table chooser pick natural_log_exp_and_others for both
    Exp and Ln (greedy first-match otherwise alternates between the exp-only
    and ln-only tables, costing a 1283ns table reload per switch). Table list
    positions are preserved; only the claimed function sets shrink."""
    if _STATE.get("tables_patched"):
        return
    import concourse.bacc as bacc
    import concourse.mybir as mybir

    orig = bacc.get_activation_tables
    keep = "natural_log_exp_and_others"
    drop = {mybir.ActivationFunctionType.Exp, mybir.ActivationFunctionType.Ln}

    def patched(arch):
        tabs = orig(arch)
        return {
            name: (funcs if name == keep else funcs - drop)
            for name, funcs in tabs.items()
        }

    bacc.get_activation_tables = patched
    _STATE["tables_patched"] = True


def _build_nc(loop_n=None):
    _force_combined_exp_ln_table()
    import concourse.mybir as mybir
    from concourse import bacc
    from concourse.tile import TileContext
    from contextlib import ExitStack

    f16 = mybir.dt.float16
    f32 = mybir.dt.float32
    AF = mybir.ActivationFunctionType

    nc = bacc.Bacc(
        "TRN2",
        target_bir_lowering=False,
        debug=False,
        num_devices=N_CORES,
    )

    xT = nc.dram_tensor("xT", [NTC, 128, 8, TC], f16, kind="ExternalInput")
    wqT = nc.dram_tensor("wqT", [128, 8, 512], f16, kind="ExternalInput")
    wkT = nc.dram_tensor("wkT", [128, 8, 512], f16, kind="ExternalInput")
    wvT = nc.dram_tensor("wvT", [128, 8, 512], f16, kind="ExternalInput")
    wpT = nc.dram_tensor("wpT", [128, 4, 1024], f16, kind="ExternalInput")
    cosq = nc.dram_tensor("cosq", [128, T], f16, kind="ExternalInput")
    sinq = nc.dram_tensor("sinq", [128, T], f16, kind="ExternalInput")
    cosk = nc.dram_tensor("cosk", [128, T], f16, kind="ExternalInput")
    sink = nc.dram_tensor("sink", [128, T], f16, kind="ExternalInput")
    trid = nc.dram_tensor("trid", [128, 128], f16, kind="ExternalInput")
    ocd = nc.dram_tensor("ocd", [128, 2], f16, kind="ExternalInput")
    obqd = nc.dram_tensor("obqd", [2, 128], f16, kind="ExternalInput")
    outd = nc.dram_tensor("out", [T, C], f16, kind="ExternalOutput")

    with TileContext(nc) as tc, ExitStack() as ctx:
        const = ctx.enter_context(tc.tile_pool(name="const", bufs=1))
        xpool = ctx.enter_context(tc.tile_pool(name="xp", bufs=2))
        persist = ctx.enter_context(tc.tile_pool(name="persist", bufs=1))
        work = ctx.enter_context(tc.tile_pool(name="work", bufs=4))
        attp = ctx.enter_context(tc.tile_pool(name="attp", bufs=6))
        outp = ctx.enter_context(tc.tile_pool(name="outp", bufs=2))
        psA = ctx.enter_context(tc.tile_pool(name="psA", bufs=2, space="PSUM"))
        psB = ctx.enter_context(tc.tile_pool(name="psB", bufs=2, space="PSUM"))
        psY = ctx.enter_context(tc.tile_pool(name="psY", bufs=2, space="PSUM"))
        psS = ctx.enter_context(tc.tile_pool(name="psS", bufs=2, space="PSUM"))

        # ---- constants (DMA'd in first-use order) ----
        wq_sb = const.tile([128, 8, 512], f16, tag="wq")
        wk_sb = const.tile([128, 8, 512], f16, tag="wk")
        for ci in range(8):  # split so the first matmuls start at 1/8 loaded
            nc.sync.dma_start(wq_sb[:, ci, :], wqT[:, ci, :])
            nc.sync.dma_start(wk_sb[:, ci, :], wkT[:, ci, :])
        cosq_sb = const.tile([128, T], f16, tag="cosq")
        nc.sync.dma_start(cosq_sb, cosq[:, :])
        sinq_sb = const.tile([128, T], f16, tag="sinq")
        nc.sync.dma_start(sinq_sb, sinq[:, :])
        cosk_sb = const.tile([128, T], f16, tag="cosk")
        nc.sync.dma_start(cosk_sb, cosk[:, :])
        sink_sb = const.tile([128, T], f16, tag="sink")
        nc.sync.dma_start(sink_sb, sink[:, :])
        oc_sb = const.tile([128, 2], f16, tag="oc")
        nc.sync.dma_start(oc_sb, ocd[:, :])
        obq_sb = const.tile([2, 128], f16, tag="obq")
        nc.sync.dma_start(obq_sb, obqd[:, :])
        wv_sb = const.tile([128, 8, 512], f16, tag="wv")
        nc.sync.dma_start(wv_sb, wvT[:, :, :])
        tri_sb = const.tile([128, 128], f16, tag="tri")
        nc.sync.dma_start(tri_sb, trid[:, :])
        wp_sb = const.tile([128, 4, 1024], f16, tag="wp")
        nc.sync.dma_start(wp_sb, wpT[:, :, :])
        bsh = const.tile([128, 1], f32, tag="bsh")
        nc.vector.memset(bsh, LOG_SHRINK)

        # ---- persistent activations ----
        qT = [
            persist.tile([128, T], f16, tag=f"qT{p}", name=f"qT{p}")
            for p in range(4)
        ]
        kT = [
            persist.tile([128, T], f16, tag=f"kT{p}", name=f"kT{p}")
            for p in range(4)
        ]
        yT = [
            persist.tile([128, T], f16, tag=f"yT{p}", name=f"yT{p}")
            for p in range(4)
        ]
        v_sb = persist.tile([128, NST, HPC, 65], f16, tag="v")
        nc.vector.memset(v_sb[:, :, :, 64:65], 1.0)
        # rkT[s_part, si, h] = ss_k^-1/2 (= 0.125/rms_k); f32 for exp scale AP
        rkT = persist.tile([128, NST, HPC], f32, tag="rkT")

        xts = {}

        def dma_x(tci, split=False):
            # issued from the (idle) Pool engine's DGE queue so x loads run
            # in parallel with the big const DMAs on the sync queue
            xt = xpool.tile([128, 8, TC], f16, tag="x", name=f"xt{tci}")
            if split:
                for ci in range(8):
                    nc.gpsimd.dma_start(xt[:, ci, :], xT[tci, :, ci, :])
            else:
                nc.gpsimd.dma_start(xt, xT[tci])
            xts[tci] = xt

        def gen_a_pair(tci, p):
            """Projection + RoPE + stats for head pair p of chunk tci.
            Yields after each atomic step (~1 PE matmul of work)."""
            xt = xts[tci]
            tsl = slice(tci * TC, (tci + 1) * TC)
            ps_q = psA.tile([128, TC], f32, tag="kqv", name="ps_q")
            for ci in range(8):
                nc.tensor.matmul(
                    ps_q,
                    lhsT=wq_sb[:, ci, p * 128 : (p + 1) * 128],
                    rhs=xt[:, ci, :],
                    start=(ci == 0),
                    stop=(ci == 7),
                )
                yield
            qraw = work.tile([128, TC], f16, tag="qraw")
            nc.vector.tensor_copy(qraw, ps_q)
            sq_q = work.tile([128, TC], f16, tag="sq_q")
            nc.vector.tensor_mul(sq_q, qraw, qraw)
            yield
            # q stats: colsums of sq_q per head -> [2, TC]
            ss = psS.tile([128, TC], f32, tag="s", name="ss_q")
            nc.tensor.matmul(ss[0:2, :], lhsT=oc_sb, rhs=sq_q, start=True, stop=True)
            # q norm scalars: rr_q = (ss_q/64)^-0.5 via Ln+Exp, f16 for bc matmul
            lnq = work.tile([2, TC], f32, tag="lnq")
            nc.scalar.activation(lnq, ss[0:2, :], AF.Ln, scale=1.0 / 64.0)
            rrq = work.tile([2, TC], f16, tag="rrq")
            with nc.allow_low_precision(reason="1/rms ~1 fits fp16"):
                nc.scalar.activation(rrq, lnq, AF.Exp, scale=-0.5)
            yield
            bc = psS.tile([128, TC], f32, tag="s", name="bc_q")
            nc.tensor.matmul(bc, lhsT=obq_sb, rhs=rrq, start=True, stop=True)
            yield
            # RoPE q: interleaved d-layout makes rotate-half a partition swap
            # (signs and w_q folded into the sinq/cosq tables)
            t1q = work.tile([128, TC], f16, tag="t1q")
            nc.vector.tensor_mul(t1q, qraw, cosq_sb[:, tsl])
            sh_q = work.tile([128, TC], f16, tag="sh_q")
            nc.vector.stream_shuffle(sh_q, qraw, _SWAP_MASK)
            qshq = work.tile([128, TC], f16, tag="qshq")
            nc.vector.tensor_mul(qshq, sh_q, sinq_sb[:, tsl])
            t2q = work.tile([128, TC], f16, tag="t2q")
            nc.vector.tensor_add(t2q, t1q, qshq)
            nc.vector.tensor_mul(qT[p][:, tsl], t2q, bc)
            yield
            ps_k = psA.tile([128, TC], f32, tag="kqv", name="ps_k")
            for ci in range(8):
                nc.tensor.matmul(
                    ps_k,
                    lhsT=wk_sb[:, ci, p * 128 : (p + 1) * 128],
                    rhs=xt[:, ci, :],
                    start=(ci == 0),
                    stop=(ci == 7),
                )
                yield
            kraw = work.tile([128, TC], f16, tag="kraw")
            nc.vector.tensor_copy(kraw, ps_k)
            sq_k = work.tile([128, TC], f16, tag="sq_k")
            nc.vector.tensor_mul(sq_k, kraw, kraw)
            yield
            # k stats, transposed: ssT[s_part, st, h] via 4 tiny matmuls
            ss2 = psS.tile([128, TC], f32, tag="s", name="ss_k")
            for st in range(4):
                nc.tensor.matmul(
                    ss2[:, 2 * st : 2 * st + 2],
                    lhsT=sq_k[:, st * 128 : (st + 1) * 128],
                    rhs=oc_sb,
                    start=True,
                    stop=True,
                )
            # rkT = ss_k^-0.5 = exp(-0.5 ln(ss_k))  [0.125/sqrt(64) folded exactly]
            lnk = work.tile([128, 4, 2], f32, tag="lnk")
            nc.scalar.activation(
                lnk, ss2[:, 0:8].rearrange("p (a b) -> p a b", b=2), AF.Ln
            )
            nc.scalar.activation(
                rkT[:, tci * 4 : tci * 4 + 4, 2 * p : 2 * p + 2],
                lnk,
                AF.Exp,
                scale=-0.5,
            )
            yield
            # RoPE k (no norm here; k-norm applied via exp scale)
            t1k = work.tile([128, TC], f16, tag="t1k")
            nc.vector.tensor_mul(t1k, kraw, cosk_sb[:, tsl])
            sh_k = work.tile([128, TC], f16, tag="sh_k")
            nc.vector.stream_shuffle(sh_k, kraw, _SWAP_MASK)
            qshk = work.tile([128, TC], f16, tag="qshk")
            nc.vector.tensor_mul(qshk, sh_k, sink_sb[:, tsl])
            nc.vector.tensor_add(kT[p][:, tsl], t1k, qshk)
            yield

        def gen_a_v(tci, st):
            """v projection for subtile st of chunk tci."""
            xt = xts[tci]
            pv = psA.tile([128, TC], f32, tag="kqv", name="pv")
            for ci in range(8):
                nc.tensor.matmul(
                    pv,
                    lhsT=xt[:, ci, st * 128 : (st + 1) * 128],
                    rhs=wv_sb[:, ci, :],
                    start=(ci == 0),
                    stop=(ci == 7),
                )
                yield
            nc.vector.tensor_copy(
                v_sb[:, tci * 4 + st, :, 0:64],
                pv.rearrange("p (h d) -> p h d", h=HPC),
            )
            yield

        def gen_c_unit(tci, st_local, co):
            """Output projection for token tile st of chunk tci, half co."""
            st = tci * 4 + st_local
            po = psA.tile([128, TC], f32, tag="kqv", name="po")
            for p in range(4):
                nc.tensor.matmul(
                    po,
                    lhsT=yT[p][:, st * 128 : (st + 1) * 128],
                    rhs=wp_sb[:, p, co * 512 : (co + 1) * 512],
                    start=(p == 0),
                    stop=(p == 3),
                )
                yield
            ot = outp.tile([128, TC], f16, tag="o")
            with nc.allow_low_precision(reason="fp16 partials; host sums in fp32"):
                nc.vector.tensor_copy(ot, po)
            nc.sync.dma_start(
                outd[st * 128 : (st + 1) * 128, co * 512 : (co + 1) * 512], ot
            )
            yield

        def emit_b_head(tci, h, pop):
            """Attention for head h over this t-chunk; pop() drains filler."""
            p, hl = h // 2, h % 2
            hsl = slice(hl * 64, (hl + 1) * 64)
            n_s = 4 * (tci + 1)
            tbase = tci * TC
            tsl = slice(tbase, tbase + TC)

            def score(si):
                d = si * 128 - tbase if si >= 4 * tci else 0
                ps_s = psB.tile([128, TC], f32, tag="sc", name="ps_s")
                nc.tensor.matmul(
                    ps_s[:, d:TC],
                    lhsT=kT[p][hsl, si * 128 : (si + 1) * 128],
                    rhs=qT[p][hsl, tbase + d : tbase + TC],
                    start=True,
                    stop=True,
                )
                return ps_s, d

            ps_y = psY.tile([128, TC], f32, tag="y", name="ps_y")
            pend = [score(0)]
            for si in range(n_s):
                ps_s, d = pend.pop(0)
                at = attp.tile([128, TC], f16, tag="at")
                with nc.allow_low_precision(reason="softmax weights fp16"):
                    nc.scalar.activation(
                        at[:, d:TC],
                        ps_s[:, d:TC],
                        AF.Exp,
                        bias=bsh,
                        scale=rkT[:, si, h : h + 1],
                    )
                if si + 1 < n_s:
                    pend.append(score(si + 1))
                if si >= 4 * tci:  # diagonal subtile: triangular mask block
                    nc.vector.tensor_mul(
                        at[:, d : d + 128], at[:, d : d + 128], tri_sb
                    )
                nc.tensor.matmul(
                    ps_y[0:65, d:TC],
                    lhsT=v_sb[:, si, h, :],
                    rhs=at[:, d:TC],
                    start=(si == 0),
                    stop=(si == n_s - 1),
                )
                pop()
            # softmax denominator: row 64 of ps_y (shrunk by 2^-10, cancels)
            rec = work.tile([1, TC], f16, tag="rec")
            with nc.allow_low_precision(reason="1/denom bounded by shrink"):
                nc.vector.reciprocal(rec, ps_y[64:65, :])
            db_sb = work.tile([64, TC], f16, tag="db")
            nc.gpsimd.partition_broadcast(db_sb, rec)
            nc.vector.tensor_mul(yT[p][hsl, tsl], ps_y[0:64, :], db_sb)
            pop()

        def body():
            # prologue: x chunk 0 + projections for chunk 0
            dma_x(0, split=True)
            for p in range(4):
                for _ in gen_a_pair(0, p):
                    pass
            for st in range(4):
                for _ in gen_a_v(0, st):
                    pass
            dma_x(1)

            from itertools import chain

            N_PAIR_STEPS, N_V_STEPS, N_C_STEPS = 23, 9, 5
            for tci in range(NTC):
                if tci + 2 < NTC:
                    dma_x(tci + 2)
                gens = []
                total_fill = 0
                if tci + 1 < NTC:
                    for p in range(4):
                        gens.append(gen_a_pair(tci + 1, p))
                        total_fill += N_PAIR_STEPS
                    for st in range(4):
                        gens.append(gen_a_v(tci + 1, st))
                        total_fill += N_V_STEPS
                if tci > 0:
                    for st_local in range(4):
                        for co in range(2):
                            gens.append(gen_c_unit(tci - 1, st_local, co))
                            total_fill += N_C_STEPS
                fill_it = chain(*gens)

                steps_total = 8 * (4 * (tci + 1) + 1)
                state = {"step": 0, "emitted": 0}

                def pop():
                    state["step"] += 1
                    target = (state["step"] * total_fill + steps_total - 1) // steps_total
                    while state["emitted"] < target:
                        try:
                            next(fill_it)
                        except StopIteration:
                            state["emitted"] = total_fill
                            break
                        state["emitted"] += 1

                for h in range(HPC):
                    emit_b_head(tci, h, pop)
                # drain leftover filler
                for _ in fill_it:
                    pass

            # epilogue: output projection for last chunk
            for st_local in range(4):
                for co in range(2):
                    for _ in gen_c_unit(NTC - 1, st_local, co):
                        pass

        if loop_n is None:
            body()
        else:
            with tc.For_i(0, loop_n, 1):
                body()

    return nc


def _get_nc(loop_n=None):
    key = ("nc", loop_n)
    if key not in _STATE:
        nc = _build_nc(loop_n)
        nc.finalize()
        _STATE[key] = nc
    return _STATE[key]


def _d_order():
    """Interleaved head-dim order: position 2j holds dim j, 2j+1 holds dim j+32,
    so rotate-half is a swap of adjacent partitions (stream_shuffle)."""
    order = np.empty(D, dtype=np.int64)
    order[0::2] = np.arange(32)
    order[1::2] = np.arange(32) + 32
    return order


def _rope_tables(w):
    """cosW/sinW [128, T] f16 in interleaved d-order, norm weight and the
    rotate-half signs folded in."""
    inv_freq = 1.0 / (10000.0 ** (np.arange(0, D, 2, dtype=np.float64) / D))
    t_pos = np.arange(T, dtype=np.float64)
    freqs = t_pos[:, None] * inv_freq[None, :]  # [T, 32]
    f2 = np.concatenate([freqs, freqs], axis=-1)  # [T, 64]
    w = np.asarray(w, dtype=np.float64)  # [64]
    order = _d_order()
    sign = np.where(order < 32, -1.0, 1.0)  # rot[d] = -x[d+32] (d<32), +x[d-32]
    cosB = w[:, None] * np.cos(f2).T  # [64, T], plain d order
    sinB = w[:, None] * np.sin(f2).T
    cosT = cosB[order].astype(np.float16)
    sinT = (sign[:, None] * sinB[order]).astype(np.float16)
    cos2 = np.concatenate([cosT, cosT], axis=0)  # [128, T]
    sin2 = np.concatenate([sinT, sinT], axis=0)
    return np.ascontiguousarray(cos2), np.ascontiguousarray(sin2)


def _prep_inputs(x, W_kqv, W_proj, q_norm_w, k_norm_w):
    x = np.asarray(x, dtype=np.float32)
    W_kqv = np.asarray(W_kqv, dtype=np.float32)
    W_proj = np.asarray(W_proj, dtype=np.float32)
    q_norm_w = np.asarray(q_norm_w, dtype=np.float32)
    k_norm_w = np.asarray(k_norm_w, dtype=np.float32)

    cosq, sinq = _rope_tables(q_norm_w)
    cosk, sink = _rope_tables(k_norm_w)

    # triangular mask for the diagonal 128-block: keep t >= s
    si = np.arange(128)[:, None]
    cj = np.arange(128)[None, :]
    tri = (cj >= si).astype(np.float16)

    # oc: colsum selectors per head half
    oc = np.zeros((128, 2), dtype=np.float16)
    oc[0:64, 0] = 1.0
    oc[64:128, 1] = 1.0

    # obq: expand rr rows to 64-partition halves (pure ones; w in tables)
    obq = np.zeros((2, 128), dtype=np.float16)
    obq[0, 0:64] = 1.0
    obq[1, 64:128] = 1.0

    order = _d_order()

    def wt_kqv(rows, perm=False):
        # rows: [512, 1024] -> lhsT layout [128, 8, 512] fp16
        if perm:  # interleave d within each head (q/k only)
            rows = rows.reshape(8, D, C)[:, order, :].reshape(512, C)
        wT = rows.T.astype(np.float16)  # [1024, 512]
        return np.ascontiguousarray(wT.reshape(8, 128, 512).transpose(1, 0, 2))

    Wk, Wq, Wv = W_kqv[0:C], W_kqv[C : 2 * C], W_kqv[2 * C : 3 * C]

    in_maps = []
    for c in range(N_CORES):
        b, g = c // 2, c % 2
        rs = slice(512 * g, 512 * (g + 1))
        xTb = x[b].T.astype(np.float16)  # [C, T]
        xTr = np.ascontiguousarray(
            xTb.reshape(8, 128, NTC, TC).transpose(2, 1, 0, 3)
        )  # [NTC, 128, 8, TC]
        wp = W_proj[:, rs].T.astype(np.float16)  # [512, 1024]
        wpr = np.ascontiguousarray(wp.reshape(4, 128, 1024).transpose(1, 0, 2))
        in_maps.append(
            {
                "xT": xTr,
                "wqT": wt_kqv(Wq[rs], perm=True),
                "wkT": wt_kqv(Wk[rs], perm=True),
                "wvT": wt_kqv(Wv[rs]),
                "wpT": wpr,
                "cosq": cosq,
                "sinq": sinq,
                "cosk": cosk,
                "sink": sink,
                "trid": tri,
                "ocd": oc,
                "obqd": obq,
            }
        )
    return in_maps


def _get_runner(loop_n=None):
    """Build (once) a cached jitted SPMD runner mirroring
    bass2jax.run_bass_via_pjrt, so repeated calls reuse the compiled NEFF."""
    key = ("runner", loop_n)
    if key in _STATE:
        return _STATE[key]

    import jax
    import concourse.mybir as mybir
    from concourse import bass2jax
    from concourse.bass2jax import _bass_exec_p, partition_id_tensor
    from jax.experimental.shard_map import shard_map
    from jax.sharding import Mesh, NamedSharding, PartitionSpec

    bass2jax.install_neuronx_cc_hook()
    nc = _get_nc(loop_n)

    partition_name = nc.partition_id_tensor.name if nc.partition_id_tensor else None
    in_names, out_names, out_avals, zero_outs = [], [], [], []
    for alloc in nc.m.functions[0].allocations:
        if not isinstance(alloc, mybir.MemoryLocationSet):
            continue
        name = alloc.memorylocations[0].name
        if alloc.kind == "ExternalInput":
            if name != partition_name:
                in_names.append(name)
        elif alloc.kind == "ExternalOutput":
            shape = tuple(alloc.tensor_shape)
            dtype = mybir.dt.np(alloc.dtype)
            out_names.append(name)
            out_avals.append(jax.core.ShapedArray(shape, dtype))
            zero_outs.append(np.zeros(shape, dtype))
    n_params = len(in_names)
    all_names = in_names + out_names
    if partition_name is not None:
        all_names.append(partition_name)

    def _body(*args):
        operands = list(args)
        if partition_name is not None:
            operands.append(partition_id_tensor())
        outs = _bass_exec_p.bind(
            *operands,
            out_avals=tuple(out_avals),
            in_names=tuple(all_names),
            out_names=tuple(out_names),
            lowering_input_output_aliases=(),
            sim_require_finite=True,
            sim_require_nnan=True,
            nc=nc,
        )
        return tuple(outs)

    devices = jax.devices()[:N_CORES]
    mesh = Mesh(np.asarray(devices), ("core",))
    spec = PartitionSpec("core")
    n_outs = len(out_names)
    sharded = jax.jit(
        shard_map(
            _body,
            mesh=mesh,
            in_specs=(spec,) * (n_params + n_outs),
            out_specs=(spec,) * n_outs,
            check_rep=False,
        ),
        keep_unused=True,
    )
    sharding = NamedSharding(mesh, spec)
    zeros_dev = [
        jax.device_put(
            np.zeros((N_CORES * z.shape[0], *z.shape[1:]), z.dtype), sharding
        )
        for z in zero_outs
    ]
    runner = {
        "sharded": sharded,
        "in_names": in_names,
        "out_names": out_names,
        "out_avals": out_avals,
        "zeros_dev": zeros_dev,
        "sharding": sharding,
    }
    _STATE[key] = runner
    return runner


def _concat_inputs(in_maps, runner):
    return [
        np.concatenate([np.asarray(in_maps[c][n]) for c in range(N_CORES)], axis=0)
        for n in runner["in_names"]
    ]


def _execute(in_maps):
    """Returns list (per core) of {out_name: np.ndarray}."""
    runner = _get_runner()
    concat_in = _concat_inputs(in_maps, runner)
    out_arrs = runner["sharded"](*concat_in, *runner["zeros_dev"])
    return [
        {
            n: np.asarray(out_arrs[i]).reshape(
                N_CORES, *runner["out_avals"][i].shape
            )[c]
            for i, n in enumerate(runner["out_names"])
        }
        for c in range(N_CORES)
    ]


def _wall(runner, in_maps, iters):
    import time
    import jax

    concat_in = [
        jax.device_put(a, runner["sharding"])
        for a in _concat_inputs(in_maps, runner)
    ]
    args = (*concat_in, *runner["zeros_dev"])
    jax.block_until_ready(runner["sharded"](*args))  # warmup
    times = []
    for _ in range(iters):
        t0 = time.perf_counter()
        jax.block_until_ready(runner["sharded"](*args))
        times.append(time.perf_counter() - t0)
    times.sort()
    return times


def _timed(in_maps, iters=20, n_lo=1, n_hi=33):
    """Per-pass HW time via two device-side repeat counts: the dispatch/tunnel
    overhead cancels in the difference."""
    r_lo = _get_runner(None if n_lo == 1 else n_lo)
    r_hi = _get_runner(n_hi)
    t_lo = _wall(r_lo, in_maps, iters)
    t_hi = _wall(r_hi, in_maps, iters)
    k = max(3, iters // 4)
    lo = sum(t_lo[:k]) / k
    hi = sum(t_hi[:k]) / k
    per_pass = (hi - lo) / (n_hi - n_lo)
    return per_pass, lo, hi


def kernel(**inputs):
    in_maps = _prep_inputs(**inputs)
    res = _execute(in_maps)
    out = np.zeros((B, T, C), dtype=np.float32)
    for c in range(N_CORES):
        out[c // 2] += res[c]["out"].astype(np.float32)
    return out


# revision 9
# speedup vs baseline: 1.1608x; 1.0661x over previous
"""Multi-head causal attention (RoPE + per-head RMSNorm) on 8 TRN2 NeuronCores.

v2: activation-table churn removed (Ln/Exp only on ACT engine), k-norm folded
into exp's per-partition scale AP, denominator via fp32 reciprocal + exp bias
shrink, partial-column diagonal tiles, software-pipelined emission that keeps
the in-order PE queue fed (score(si+1) before AV(si), next-chunk projections
and prev-chunk output projection interleaved at si granularity).

Sharding: core c -> batch b = c//2, head group g = c%2 (heads 8g..8g+8).
Each core computes a partial out[b] over its 8 heads' channels; host sums the
two partials per batch.
"""

import sys

import numpy as np

sys.path.insert(0, "/opt/trn_rl_repo")

B, T, C, H, D = 4, 2048, 1024, 16, 64
N_CORES = 8
HPC = H // 2  # heads per core: 8
TC = 512  # t-chunk (matmul free dim)
NTC = T // TC  # 4
NST = T // 128  # 16 s/t subtiles
LOG_SHRINK = -6.931471805599453  # ln(2^-10): at' shrink so unnormalized y fits fp16

_STATE: dict = {}
FILL_FRAC = 1.0

# within each 32-partition quadrant: swap adjacent pairs (2j <-> 2j+1)
_SWAP_MASK = [j + 1 if j % 2 == 0 else j - 1 for j in range(32)]


def _force_combined_exp_ln_table():
    """Make the act-table chooser pick natural_log_exp_and_others for both
    Exp and Ln (greedy first-match otherwise alternates between the exp-only
    and ln-only tables, costing a 1283ns table reload per switch). Table list
    positions are preserved; only the claimed function sets shrink."""
    if _STATE.get("tables_patched"):
        return
    import concourse.bacc as bacc
    import concourse.mybir as mybir

    orig = bacc.get_activation_tables
    keep = "natural_log_exp_and_others"
    drop = {mybir.ActivationFunctionType.Exp, mybir.ActivationFunctionType.Ln}

    def patched(arch):
        tabs = orig(arch)
        return {
            name: (funcs if name == keep else funcs - drop)
            for name, funcs in tabs.items()
        }

    bacc.get_activation_tables = patched
    _STATE["tables_patched"] = True


def _build_nc(loop_n=None):
    _force_combined_exp_ln_table()
    import concourse.mybir as mybir
    from concourse import bacc
    from concourse.tile import TileContext
    from contextlib import ExitStack

    f16 = mybir.dt.float16
    f32 = mybir.dt.float32
    AF = mybir.ActivationFunctionType

    nc = bacc.Bacc(
        "TRN2",
        target_bir_lowering=False,
        debug=False,
        num_devices=N_CORES,
    )

    xT = nc.dram_tensor("xT", [NTC, 128, 8, TC], f16, kind="ExternalInput")
    wqT = nc.dram_tensor("wqT", [128, 8, 512], f16, kind="ExternalInput")
    wkT = nc.dram_tensor("wkT", [128, 8, 512], f16, kind="ExternalInput")
    wvT = nc.dram_tensor("wvT", [128, 8, 512], f16, kind="ExternalInput")
    wpT = nc.dram_tensor("wpT", [128, 4, 1024], f16, kind="ExternalInput")
    cosq = nc.dram_tensor("cosq", [128, T], f16, kind="ExternalInput")
    sinq = nc.dram_tensor("sinq", [128, T], f16, kind="ExternalInput")
    cosk = nc.dram_tensor("cosk", [128, T], f16, kind="ExternalInput")
    sink = nc.dram_tensor("sink", [128, T], f16, kind="ExternalInput")
    trid = nc.dram_tensor("trid", [128, 128], f16, kind="ExternalInput")
    ocd = nc.dram_tensor("ocd", [128, 2], f16, kind="ExternalInput")
    obqd = nc.dram_tensor("obqd", [2, 128], f16, kind="ExternalInput")
    outd = nc.dram_tensor("out", [T, C], f16, kind="ExternalOutput")

    with TileContext(nc) as tc, ExitStack() as ctx:
        const = ctx.enter_context(tc.tile_pool(name="const", bufs=1))
        xpool = ctx.enter_context(tc.tile_pool(name="xp", bufs=2))
        persist = ctx.enter_context(tc.tile_pool(name="persist", bufs=1))
        work = ctx.enter_context(tc.tile_pool(name="work", bufs=4))
        attp = ctx.enter_context(tc.tile_pool(name="attp", bufs=8))
        outp = ctx.enter_context(tc.tile_pool(name="outp", bufs=3))
        psA = ctx.enter_context(tc.tile_pool(name="psA", bufs=2, space="PSUM"))
        psB = ctx.enter_context(tc.tile_pool(name="psB", bufs=2, space="PSUM"))
        psY = ctx.enter_context(tc.tile_pool(name="psY", bufs=2, space="PSUM"))
        psS = ctx.enter_context(tc.tile_pool(name="psS", bufs=2, space="PSUM"))

        # ---- constants (DMA'd in first-use order) ----
        wq_sb = const.tile([128, 8, 512], f16, tag="wq")
        wk_sb = const.tile([128, 8, 512], f16, tag="wk")
        for ci in range(8):  # split so the first matmuls start at 1/8 loaded
            nc.sync.dma_start(wq_sb[:, ci, :], wqT[:, ci, :])
            nc.sync.dma_start(wk_sb[:, ci, :], wkT[:, ci, :])
        cosq_sb = const.tile([128, T], f16, tag="cosq")
        nc.sync.dma_start(cosq_sb, cosq[:, :])
        sinq_sb = const.tile([128, T], f16, tag="sinq")
        nc.sync.dma_start(sinq_sb, sinq[:, :])
        cosk_sb = const.tile([128, T], f16, tag="cosk")
        nc.sync.dma_start(cosk_sb, cosk[:, :])
        sink_sb = const.tile([128, T], f16, tag="sink")
        nc.sync.dma_start(sink_sb, sink[:, :])
        oc_sb = const.tile([128, 2], f16, tag="oc")
        nc.sync.dma_start(oc_sb, ocd[:, :])
        obq_sb = const.tile([2, 128], f16, tag="obq")
        nc.sync.dma_start(obq_sb, obqd[:, :])
        wv_sb = const.tile([128, 8, 512], f16, tag="wv")
        nc.sync.dma_start(wv_sb, wvT[:, :, :])
        tri_sb = const.tile([128, 128], f16, tag="tri")
        nc.sync.dma_start(tri_sb, trid[:, :])
        wp_sb = const.tile([128, 4, 1024], f16, tag="wp")
        nc.sync.dma_start(wp_sb, wpT[:, :, :])
        bsh = const.tile([128, 1], f32, tag="bsh")
        nc.vector.memset(bsh, LOG_SHRINK)

        # ---- persistent activations ----
        qT = [
            persist.tile([128, T], f16, tag=f"qT{p}", name=f"qT{p}")
            for p in range(4)
        ]
        kT = [
            persist.tile([128, T], f16, tag=f"kT{p}", name=f"kT{p}")
            for p in range(4)
        ]
        yT = [
            persist.tile([128, T], f16, tag=f"yT{p}", name=f"yT{p}")
            for p in range(4)
        ]
        v_sb = persist.tile([128, NST, HPC, 65], f16, tag="v")
        nc.vector.memset(v_sb[:, :, :, 64:65], 1.0)
        # rkT[s_part, si, h] = ss_k^-1/2 (= 0.125/rms_k); f32 for exp scale AP
        rkT = persist.tile([128, NST, HPC], f32, tag="rkT")

        xts = {}

        def dma_x(tci, split=False):
            # issued from the (idle) Pool engine's DGE queue so x loads run
            # in parallel with the big const DMAs on the sync queue
            xt = xpool.tile([128, 8, TC], f16, tag="x", name=f"xt{tci}")
            if split:
                for ci in range(8):
                    nc.gpsimd.dma_start(xt[:, ci, :], xT[tci, :, ci, :])
            else:
                nc.gpsimd.dma_start(xt, xT[tci])
            xts[tci] = xt

        def gen_a_pair(tci, p):
            """Projection + RoPE + stats for head pair p of chunk tci.
            Yields after each atomic step (~1 PE matmul of work)."""
            xt = xts[tci]
            tsl = slice(tci * TC, (tci + 1) * TC)
            ps_q = psA.tile([128, TC], f32, tag="kqv", name="ps_q")
            for ci in range(8):
                nc.tensor.matmul(
                    ps_q,
                    lhsT=wq_sb[:, ci, p * 128 : (p + 1) * 128],
                    rhs=xt[:, ci, :],
                    start=(ci == 0),
                    stop=(ci == 7),
                )
                yield
            qraw = work.tile([128, TC], f16, tag="qraw")
            nc.vector.tensor_copy(qraw, ps_q)
            sq_q = work.tile([128, TC], f16, tag="sq_q")
            nc.vector.tensor_mul(sq_q, qraw, qraw)
            yield
            # q stats: colsums of sq_q per head -> [2, TC]
            ss = psS.tile([128, TC], f32, tag="s", name="ss_q")
            nc.tensor.matmul(ss[0:2, :], lhsT=oc_sb, rhs=sq_q, start=True, stop=True)
            # q norm scalars: rr_q = (ss_q/64)^-0.5 via Ln+Exp, f16 for bc matmul
            lnq = work.tile([2, TC], f32, tag="lnq")
            nc.scalar.activation(lnq, ss[0:2, :], AF.Ln, scale=1.0 / 64.0)
            rrq = work.tile([2, TC], f16, tag="rrq")
            with nc.allow_low_precision(reason="1/rms ~1 fits fp16"):
                nc.scalar.activation(rrq, lnq, AF.Exp, scale=-0.5)
            yield
            bc = psS.tile([128, TC], f32, tag="s", name="bc_q")
            nc.tensor.matmul(bc, lhsT=obq_sb, rhs=rrq, start=True, stop=True)
            yield
            # RoPE q: interleaved d-layout makes rotate-half a partition swap
            # (signs and w_q folded into the sinq/cosq tables)
            t1q = work.tile([128, TC], f16, tag="t1q")
            nc.vector.tensor_mul(t1q, qraw, cosq_sb[:, tsl])
            sh_q = work.tile([128, TC], f16, tag="sh_q")
            nc.vector.stream_shuffle(sh_q, qraw, _SWAP_MASK)
            qshq = work.tile([128, TC], f16, tag="qshq")
            nc.vector.tensor_mul(qshq, sh_q, sinq_sb[:, tsl])
            t2q = work.tile([128, TC], f16, tag="t2q")
            nc.vector.tensor_add(t2q, t1q, qshq)
            nc.vector.tensor_mul(qT[p][:, tsl], t2q, bc)
            yield
            ps_k = psA.tile([128, TC], f32, tag="kqv", name="ps_k")
            for ci in range(8):
                nc.tensor.matmul(
                    ps_k,
                    lhsT=wk_sb[:, ci, p * 128 : (p + 1) * 128],
                    rhs=xt[:, ci, :],
                    start=(ci == 0),
                    stop=(ci == 7),
                )
                yield
            kraw = work.tile([128, TC], f16, tag="kraw")
            nc.vector.tensor_copy(kraw, ps_k)
            sq_k = work.tile([128, TC], f16, tag="sq_k")
            nc.vector.tensor_mul(sq_k, kraw, kraw)
            yield
            # k stats, transposed: ssT[s_part, st, h] via 4 tiny matmuls
            ss2 = psS.tile([128, TC], f32, tag="s", name="ss_k")
            for st in range(4):
                nc.tensor.matmul(
                    ss2[:, 2 * st : 2 * st + 2],
                    lhsT=sq_k[:, st * 128 : (st + 1) * 128],
                    rhs=oc_sb,
                    start=True,
                    stop=True,
                )
            # rkT = ss_k^-0.5 = exp(-0.5 ln(ss_k))  [0.125/sqrt(64) folded exactly]
            lnk = work.tile([128, 4, 2], f32, tag="lnk")
            nc.scalar.activation(
                lnk, ss2[:, 0:8].rearrange("p (a b) -> p a b", b=2), AF.Ln
            )
            nc.scalar.activation(
                rkT[:, tci * 4 : tci * 4 + 4, 2 * p : 2 * p + 2],
                lnk,
                AF.Exp,
                scale=-0.5,
            )
            yield
            # RoPE k (no norm here; k-norm applied via exp scale)
            t1k = work.tile([128, TC], f16, tag="t1k")
            nc.vector.tensor_mul(t1k, kraw, cosk_sb[:, tsl])
            sh_k = work.tile([128, TC], f16, tag="sh_k")
            nc.vector.stream_shuffle(sh_k, kraw, _SWAP_MASK)
            qshk = work.tile([128, TC], f16, tag="qshk")
            nc.vector.tensor_mul(qshk, sh_k, sink_sb[:, tsl])
            nc.vector.tensor_add(kT[p][:, tsl], t1k, qshk)
            yield

        def gen_a_v(tci, st):
            """v projection for subtile st of chunk tci."""
            xt = xts[tci]
            pv = psA.tile([128, TC], f32, tag="kqv", name="pv")
            for ci in range(8):
                nc.tensor.matmul(
                    pv,
                    lhsT=xt[:, ci, st * 128 : (st + 1) * 128],
                    rhs=wv_sb[:, ci, :],
                    start=(ci == 0),
                    stop=(ci == 7),
                )
                yield
            nc.vector.tensor_copy(
                v_sb[:, tci * 4 + st, :, 0:64],
                pv.rearrange("p (h d) -> p h d", h=HPC),
            )
            yield

        def gen_c_unit(tci, st_local, co):
            """Output projection for token tile st of chunk tci, half co."""
            st = tci * 4 + st_local
            po = psA.tile([128, TC], f32, tag="kqv", name="po")
            for p in range(4):
                nc.tensor.matmul(
                    po,
                    lhsT=yT[p][:, st * 128 : (st + 1) * 128],
                    rhs=wp_sb[:, p, co * 512 : (co + 1) * 512],
                    start=(p == 0),
                    stop=(p == 3),
                )
                yield
            ot = outp.tile([128, TC], f16, tag="o")
            with nc.allow_low_precision(reason="fp16 partials; host sums in fp32"):
                nc.vector.tensor_copy(ot, po)
            nc.sync.dma_start(
                outd[st * 128 : (st + 1) * 128, co * 512 : (co + 1) * 512], ot
            )
            yield

        def emit_b_head(tci, h, pop):
            """Attention for head h over this t-chunk; pop() drains filler."""
            p, hl = h // 2, h % 2
            hsl = slice(hl * 64, (hl + 1) * 64)
            n_s = 4 * (tci + 1)
            tbase = tci * TC
            tsl = slice(tbase, tbase + TC)

            def score(si):
                d = si * 128 - tbase if si >= 4 * tci else 0
                ps_s = psB.tile([128, TC], f32, tag="sc", name="ps_s")
                nc.tensor.matmul(
                    ps_s[:, d:TC],
                    lhsT=kT[p][hsl, si * 128 : (si + 1) * 128],
                    rhs=qT[p][hsl, tbase + d : tbase + TC],
                    start=True,
                    stop=True,
                )
                return ps_s, d

            ps_y = psY.tile([128, TC], f32, tag="y", name="ps_y")
            pend = [score(0)]
            for si in range(n_s):
                ps_s, d = pend.pop(0)
                at = attp.tile([128, TC], f16, tag="at")
                with nc.allow_low_precision(reason="softmax weights fp16"):
                    nc.scalar.activation(
                        at[:, d:TC],
                        ps_s[:, d:TC],
                        AF.Exp,
                        bias=bsh,
                        scale=rkT[:, si, h : h + 1],
                    )
                if si + 1 < n_s:
                    pend.append(score(si + 1))
                if si >= 4 * tci:  # diagonal subtile: triangular mask block
                    nc.vector.tensor_mul(
                        at[:, d : d + 128], at[:, d : d + 128], tri_sb
                    )
                nc.tensor.matmul(
                    ps_y[0:65, d:TC],
                    lhsT=v_sb[:, si, h, :],
                    rhs=at[:, d:TC],
                    start=(si == 0),
                    stop=(si == n_s - 1),
                )
                pop()
            # softmax denominator: row 64 of ps_y (shrunk by 2^-10, cancels)
            rec = work.tile([1, TC], f16, tag="rec")
            with nc.allow_low_precision(reason="1/denom bounded by shrink"):
                nc.vector.reciprocal(rec, ps_y[64:65, :])
            db_sb = work.tile([64, TC], f16, tag="db")
            nc.gpsimd.partition_broadcast(db_sb, rec)
            nc.vector.tensor_mul(yT[p][hsl, tsl], ps_y[0:64, :], db_sb)
            pop()

        def body():
            # prologue: x chunk 0 + projections for chunk 0
            dma_x(0, split=True)
            for p in range(4):
                for _ in gen_a_pair(0, p):
                    pass
            for st in range(4):
                for _ in gen_a_v(0, st):
                    pass
            dma_x(1)

            from itertools import chain

            N_PAIR_STEPS, N_V_STEPS, N_C_STEPS = 23, 9, 5
            for tci in range(NTC):
                if tci + 2 < NTC:
                    dma_x(tci + 2)
                gens = []
                total_fill = 0
                if tci + 1 < NTC:
                    for p in range(4):
                        gens.append(gen_a_pair(tci + 1, p))
                        total_fill += N_PAIR_STEPS
                    for st in range(4):
                        gens.append(gen_a_v(tci + 1, st))
                        total_fill += N_V_STEPS
                if tci > 0:
                    for st_local in range(4):
                        for co in range(2):
                            gens.append(gen_c_unit(tci - 1, st_local, co))
                            total_fill += N_C_STEPS
                fill_it = chain(*gens)

                steps_total = 8 * (4 * (tci + 1) + 1)
                state = {"step": 0, "emitted": 0}

                def pop():
                    state["step"] += 1
                    eff = max(1, int(steps_total * FILL_FRAC))
                    target = min(
                        total_fill,
                        (state["step"] * total_fill + eff - 1) // eff,
                    )
                    while state["emitted"] < target:
                        try:
                            next(fill_it)
                        except StopIteration:
                            state["emitted"] = total_fill
                            break
                        state["emitted"] += 1

                for h in range(HPC):
                    emit_b_head(tci, h, pop)
                # drain leftover filler
                for _ in fill_it:
                    pass

            # epilogue: output projection for last chunk
            for st_local in range(4):
                for co in range(2):
                    for _ in gen_c_unit(NTC - 1, st_local, co):
                        pass

        if loop_n is None:
            body()
        else:
            with tc.For_i(0, loop_n, 1):
                body()

    return nc


def _get_nc(loop_n=None):
    key = ("nc", loop_n)
    if key not in _STATE:
        nc = _build_nc(loop_n)
        nc.finalize()
        _STATE[key] = nc
    return _STATE[key]


def _d_order():
    """Interleaved head-dim order: position 2j holds dim j, 2j+1 holds dim j+32,
    so rotate-half is a swap of adjacent partitions (stream_shuffle)."""
    order = np.empty(D, dtype=np.int64)
    order[0::2] = np.arange(32)
    order[1::2] = np.arange(32) + 32
    return order


def _rope_tables(w):
    """cosW/sinW [128, T] f16 in interleaved d-order, norm weight and the
    rotate-half signs folded in."""
    inv_freq = 1.0 / (10000.0 ** (np.arange(0, D, 2, dtype=np.float64) / D))
    t_pos = np.arange(T, dtype=np.float64)
    freqs = t_pos[:, None] * inv_freq[None, :]  # [T, 32]
    f2 = np.concatenate([freqs, freqs], axis=-1)  # [T, 64]
    w = np.asarray(w, dtype=np.float64)  # [64]
    order = _d_order()
    sign = np.where(order < 32, -1.0, 1.0)  # rot[d] = -x[d+32] (d<32), +x[d-32]
    cosB = w[:, None] * np.cos(f2).T  # [64, T], plain d order
    sinB = w[:, None] * np.sin(f2).T
    cosT = cosB[order].astype(np.float16)
    sinT = (sign[:, None] * sinB[order]).astype(np.float16)
    cos2 = np.concatenate([cosT, cosT], axis=0)  # [128, T]
    sin2 = np.concatenate([sinT, sinT], axis=0)
    return np.ascontiguousarray(cos2), np.ascontiguousarray(sin2)


def _prep_inputs(x, W_kqv, W_proj, q_norm_w, k_norm_w):
    x = np.asarray(x, dtype=np.float32)
    W_kqv = np.asarray(W_kqv, dtype=np.float32)
    W_proj = np.asarray(W_proj, dtype=np.float32)
    q_norm_w = np.asarray(q_norm_w, dtype=np.float32)
    k_norm_w = np.asarray(k_norm_w, dtype=np.float32)

    cosq, sinq = _rope_tables(q_norm_w)
    cosk, sink = _rope_tables(k_norm_w)

    # triangular mask for the diagonal 128-block: keep t >= s
    si = np.arange(128)[:, None]
    cj = np.arange(128)[None, :]
    tri = (cj >= si).astype(np.float16)

    # oc: colsum selectors per head half
    oc = np.zeros((128, 2), dtype=np.float16)
    oc[0:64, 0] = 1.0
    oc[64:128, 1] = 1.0

    # obq: expand rr rows to 64-partition halves (pure ones; w in tables)
    obq = np.zeros((2, 128), dtype=np.float16)
    obq[0, 0:64] = 1.0
    obq[1, 64:128] = 1.0

    order = _d_order()

    def wt_kqv(rows, perm=False):
        # rows: [512, 1024] -> lhsT layout [128, 8, 512] fp16
        if perm:  # interleave d within each head (q/k only)
            rows = rows.reshape(8, D, C)[:, order, :].reshape(512, C)
        wT = rows.T.astype(np.float16)  # [1024, 512]
        return np.ascontiguousarray(wT.reshape(8, 128, 512).transpose(1, 0, 2))

    Wk, Wq, Wv = W_kqv[0:C], W_kqv[C : 2 * C], W_kqv[2 * C : 3 * C]

    in_maps = []
    for c in range(N_CORES):
        b, g = c // 2, c % 2
        rs = slice(512 * g, 512 * (g + 1))
        xTb = x[b].T.astype(np.float16)  # [C, T]
        xTr = np.ascontiguousarray(
            xTb.reshape(8, 128, NTC, TC).transpose(2, 1, 0, 3)
        )  # [NTC, 128, 8, TC]
        wp = W_proj[:, rs].T.astype(np.float16)  # [512, 1024]
        wpr = np.ascontiguousarray(wp.reshape(4, 128, 1024).transpose(1, 0, 2))
        in_maps.append(
            {
                "xT": xTr,
                "wqT": wt_kqv(Wq[rs], perm=True),
                "wkT": wt_kqv(Wk[rs], perm=True),
                "wvT": wt_kqv(Wv[rs]),
                "wpT": wpr,
                "cosq": cosq,
                "sinq": sinq,
                "cosk": cosk,
                "sink": sink,
                "trid": tri,
                "ocd": oc,
                "obqd": obq,
            }
        )
    return in_maps


def _get_runner(loop_n=None):
    """Build (once) a cached jitted SPMD runner mirroring
    bass2jax.run_bass_via_pjrt, so repeated calls reuse the compiled NEFF."""
    key = ("runner", loop_n)
    if key in _STATE:
        return _STATE[key]

    import jax
    import concourse.mybir as mybir
    from concourse import bass2jax
    from concourse.bass2jax import _bass_exec_p, partition_id_tensor
    from jax.experimental.shard_map import shard_map
    from jax.sharding import Mesh, NamedSharding, PartitionSpec

    bass2jax.install_neuronx_cc_hook()
    nc = _get_nc(loop_n)

    partition_name = nc.partition_id_tensor.name if nc.partition_id_tensor else None
    in_names, out_names, out_avals, zero_outs = [], [], [], []
    for alloc in nc.m.functions[0].allocations:
        if not isinstance(alloc, mybir.MemoryLocationSet):
            continue
        name = alloc.memorylocations[0].name
        if alloc.kind == "ExternalInput":
            if name != partition_name:
                in_names.append(name)
        elif alloc.kind == "ExternalOutput":
            shape = tuple(alloc.tensor_shape)
            dtype = mybir.dt.np(alloc.dtype)
            out_names.append(name)
            out_avals.append(jax.core.ShapedArray(shape, dtype))
            zero_outs.append(np.zeros(shape, dtype))
    n_params = len(in_names)
    all_names = in_names + out_names
    if partition_name is not None:
        all_names.append(partition_name)

    def _body(*args):
        operands = list(args)
        if partition_name is not None:
            operands.append(partition_id_tensor())
        outs = _bass_exec_p.bind(
            *operands,
            out_avals=tuple(out_avals),
            in_names=tuple(all_names),
            out_names=tuple(out_names),
            lowering_input_output_aliases=(),
            sim_require_finite=True,
            sim_require_nnan=True,
            nc=nc,
        )
        return tuple(outs)

    devices = jax.devices()[:N_CORES]
    mesh = Mesh(np.asarray(devices), ("core",))
    spec = PartitionSpec("core")
    n_outs = len(out_names)
    sharded = jax.jit(
        shard_map(
            _body,
            mesh=mesh,
            in_specs=(spec,) * (n_params + n_outs),
            out_specs=(spec,) * n_outs,
            check_rep=False,
        ),
        keep_unused=True,
    )
    sharding = NamedSharding(mesh, spec)
    zeros_dev = [
        jax.device_put(
            np.zeros((N_CORES * z.shape[0], *z.shape[1:]), z.dtype), sharding
        )
        for z in zero_outs
    ]
    runner = {
        "sharded": sharded,
        "in_names": in_names,
        "out_names": out_names,
        "out_avals": out_avals,
        "zeros_dev": zeros_dev,
        "sharding": sharding,
    }
    _STATE[key] = runner
    return runner


def _concat_inputs(in_maps, runner):
    return [
        np.concatenate([np.asarray(in_maps[c][n]) for c in range(N_CORES)], axis=0)
        for n in runner["in_names"]
    ]


def _execute(in_maps):
    """Returns list (per core) of {out_name: np.ndarray}."""
    runner = _get_runner()
    concat_in = _concat_inputs(in_maps, runner)
    out_arrs = runner["sharded"](*concat_in, *runner["zeros_dev"])
    return [
        {
            n: np.asarray(out_arrs[i]).reshape(
                N_CORES, *runner["out_avals"][i].shape
            )[c]
            for i, n in enumerate(runner["out_names"])
        }
        for c in range(N_CORES)
    ]


def _wall(runner, in_maps, iters):
    import time
    import jax

    concat_in = [
        jax.device_put(a, runner["sharding"])
        for a in _concat_inputs(in_maps, runner)
    ]
    args = (*concat_in, *runner["zeros_dev"])
    jax.block_until_ready(runner["sharded"](*args))  # warmup
    times = []
    for _ in range(iters):
        t0 = time.perf_counter()
        jax.block_until_ready(runner["sharded"](*args))
        times.append(time.perf_counter() - t0)
    times.sort()
    return times


def _timed(in_maps, iters=20, n_lo=1, n_hi=33):
    """Per-pass HW time via two device-side repeat counts: the dispatch/tunnel
    overhead cancels in the difference."""
    r_lo = _get_runner(None if n_lo == 1 else n_lo)
    r_hi = _get_runner(n_hi)
    t_lo = _wall(r_lo, in_maps, iters)
    t_hi = _wall(r_hi, in_maps, iters)
    k = max(3, iters // 4)
    lo = sum(t_lo[:k]) / k
    hi = sum(t_hi[:k]) / k
    per_pass = (hi - lo) / (n_hi - n_lo)
    return per_pass, lo, hi


def kernel(**inputs):
    in_maps = _prep_inputs(**inputs)
    res = _execute(in_maps)
    out = np.zeros((B, T, C), dtype=np.float32)
    for c in range(N_CORES):
        out[c // 2] += res[c]["out"].astype(np.float32)
    return out
